# revision 1
# baseline (speedup 1.0000x reference)
"""CovLoss (BCE + Dice + triple-Pearson) Trainium2 Bass kernel.

Strategy: pure data parallel over the batch dim (32 samples -> 8 cores x 4
samples). Each core streams fp16 logits/labels once and emits per-partition
partial sums; the host combines them in float64 (the "all-reduce" is a few
hundred scalars per core).

Layout/engine plan (cost-model driven):
  - HBM traffic cut to ~4.4MB/core: logits fp16 (host-clamped to <= 1-2^-11
    so Ln(1-p) never sees 0; the clamp perturbs ~0.01% of pixels), labels
    fp16, attention maps fp16.
  - ACT (bottleneck, ~16us): Ln(p) and Ln(1-p) via scale=-1/bias=1, with
    fused accum for sum(log(1-p)).
  - DVE: all big dots are single tensor_scalar/scalar_tensor_tensor passes
    (InstTensorScalarPtr hits the 4x fp16 perf mode; TensorTensor/TensorReduce
    do not) with fused accum_out: sum(M*y), sum(y*log p), sum(y*log(1-p)).
  - Pool/GPSIMD (otherwise idle): mask count sum(M) via tensor_scalar accum.
  - PE: row-pool matmuls; DVE grouped reduce on PSUM finishes the 4x4 pool.
  - Attention moments run in a "fat" layout (4 samples stacked on partitions,
    32 partitions each) so each moment is ONE accum op; per-sample values are
    recovered on the host by summing partition groups. The pooled-label tile
    is rotated into that layout through a DRAM bounce.
  - DMA issue is spread over SP and PE sequencers (the issuing SEQ is held
    until HWDGE accepts the copy), never ACT/DVE.
"""

import numpy as np

import concourse.bass as bass
import concourse.bacc as bacc
import concourse.tile as tile
from concourse import mybir
from concourse.bass_utils import run_bass_kernel_spmd

N_CORES = 8
N = 32
S_PER_CORE = N // N_CORES  # 4 samples per core
H = W = 512
P = 128          # SBUF partitions
T = H // P       # 4 row blocks per sample
FD = T * W       # 2048 free elements per partition per sample
N2 = H // 4      # 128 pooled size
K = N2 * N2      # 16384 elements per attention map
PPS = P // S_PER_CORE  # 32 partitions per sample in the fat layout

F32 = mybir.dt.float32
F16 = mybir.dt.float16

# stats tile columns [128, 32] fp32
C_LQ = 0          # +s     : sum(log(1-p)) per sample (ACT accum)
C_SQ = 4          # +3s+{0,1,2}: sum(M*y), sum(y*log p), sum(y*log(1-p))
C_M = 16          # +s     : sum(M) per sample (Pool accum)
C_V, C_H, C_VH, C_V2, C_H2, C_VL, C_HL, C_VHL, C_L2, C_L = range(20, 30)
STATS_W = 32


def _build_nc():
    nc = bacc.Bacc(trn_type="TRN2")

    d_logits = nc.dram_tensor("logits", [S_PER_CORE, P, FD], F16,
                              kind="ExternalInput")
    d_labels = nc.dram_tensor("labels", [S_PER_CORE, P, FD], F16,
                              kind="ExternalInput")
    d_att = nc.dram_tensor("att", [P, 2 * S_PER_CORE * N2], F16,
                           kind="ExternalInput")
    d_pool = nc.dram_tensor("poolmat", [P, T * P], F16, kind="ExternalInput")
    d_ident = nc.dram_tensor("ident", [P, P], F16, kind="ExternalInput")

    d_lpool = nc.dram_tensor("lpool", [S_PER_CORE, PPS, T, N2], F16,
                             kind="Internal")
    d_stats = nc.dram_tensor("stats", [P, STATS_W], F32,
                             kind="ExternalOutput")
    d_stats_act = nc.dram_tensor("stats_act", [P, S_PER_CORE], F32,
                                 kind="ExternalOutput")

    with tile.TileContext(nc) as tc:
        with (
            tc.tile_pool(name="consts", bufs=1) as consts,
            tc.tile_pool(name="big", bufs=2) as big,
            tc.tile_pool(name="junk", bufs=1) as junkp,
            tc.tile_pool(name="psum", bufs=2, space="PSUM") as psump,
        ):
            stats = consts.tile([P, STATS_W], F32)
            stats_act = consts.tile([P, S_PER_CORE], F32)
            ident = consts.tile([P, P], F16)
            attm = consts.tile([P, 2, S_PER_CORE * N2], F16)
            poolm = consts.tile([P, T, P], F16)
            lpool = consts.tile([P, S_PER_CORE, N2], F16)
            lfat = consts.tile([P, S_PER_CORE * N2], F16)
            vh = consts.tile([P, S_PER_CORE * N2], F16)
            junkf = consts.tile([P, S_PER_CORE * N2], F16)
            junk1 = junkp.tile([P, FD], F16, tag="junk1")
            junk2 = junkp.tile([P, FD], F16, tag="junk2")
            junkg = junkp.tile([P, FD], F16, tag="junkg")

            vt = attm[:, 0, :]
            ht = attm[:, 1, :]

            for s in range(S_PER_CORE):
                pt = big.tile([P, FD], F16, tag="p")
                nc.sync.dma_start(out=pt, in_=d_logits[s])
                yt = big.tile([P, T, W], F16, tag="y")
                nc.sync.dma_start(
                    out=yt, in_=d_labels[s].rearrange("p (t w) -> p t w", t=T))
                if s == 0:
                    nc.scalar.dma_start(out=attm,
                                        in_=d_att.rearrange("p (q f) -> p q f",
                                                            q=2))
                    nc.scalar.dma_start(
                        out=poolm, in_=d_pool.rearrange("p (t m) -> p t m",
                                                        t=T))
                    nc.scalar.dma_start(out=ident, in_=d_ident[:, :])
                ytf = yt.rearrange("p t w -> p (t w)")

                # ACT: logs (bottleneck engine - nothing else runs here)
                lp = big.tile([P, FD], F16, tag="lp")
                nc.scalar.activation(
                    out=lp, in_=pt, func=mybir.ActivationFunctionType.Ln)
                lq = big.tile([P, FD], F16, tag="lq")
                nc.scalar.activation(
                    out=lq, in_=pt, func=mybir.ActivationFunctionType.Ln,
                    scale=-1.0, bias=1.0,
                    accum_out=stats_act[:, s:s + 1])

                # DVE: fused dot products (4x bf16 perf mode + accum)
                c = C_SQ + 3 * s
                nc.vector.scalar_tensor_tensor(
                    out=junk1, in0=pt, scalar=0.4, in1=ytf,
                    op0=mybir.AluOpType.is_gt, op1=mybir.AluOpType.mult,
                    accum_out=stats[:, c:c + 1])
                nc.vector.scalar_tensor_tensor(
                    out=junk2, in0=lp, scalar=1.0, in1=ytf,
                    op0=mybir.AluOpType.mult, op1=mybir.AluOpType.mult,
                    accum_out=stats[:, c + 1:c + 2])
                nc.vector.scalar_tensor_tensor(
                    out=junk1, in0=lq, scalar=1.0, in1=ytf,
                    op0=mybir.AluOpType.mult, op1=mybir.AluOpType.mult,
                    accum_out=stats[:, c + 2:c + 3])

                # mask count (GPSIMD rejects TensorScalarPtr; DVE has room)
                nc.vector.tensor_scalar(
                    out=junkg, in0=pt, scalar1=0.4, scalar2=None,
                    op0=mybir.AluOpType.is_gt, op1=mybir.AluOpType.add,
                    accum_out=stats[:, C_M + s:C_M + s + 1])

                # PE row-pool matmuls -> PSUM [128, 512]
                ps_pool = psump.tile([P, W], F32, tag="pool")
                for t in range(T):
                    nc.tensor.matmul(
                        ps_pool, lhsT=poolm[:, t, :], rhs=yt[:, t, :],
                        start=(t == 0), stop=(t == T - 1))
                # DVE: finish 4x4 pooling (column groups of 4). bf16 out is
                # safe: 4-term sums, and every consumer tolerates 0.4% noise.
                with nc.allow_low_precision(reason="4-term pooled sums"):
                    nc.vector.tensor_reduce(
                        out=lpool[:, s, :],
                        in_=ps_pool.rearrange("p (g f) -> p g f", f=4),
                        axis=mybir.AxisListType.X, op=mybir.AluOpType.add)

                if s == 0:
                    # attention moments that need only v,h (early, on DVE)
                    nc.vector.scalar_tensor_tensor(
                        out=vh, in0=vt, scalar=1.0, in1=ht,
                        op0=mybir.AluOpType.mult, op1=mybir.AluOpType.mult,
                        accum_out=stats[:, C_VH:C_VH + 1])
                    nc.vector.scalar_tensor_tensor(
                        out=junkf, in0=vt, scalar=1.0, in1=vt,
                        op0=mybir.AluOpType.mult, op1=mybir.AluOpType.mult,
                        accum_out=stats[:, C_V2:C_V2 + 1])
                    nc.vector.scalar_tensor_tensor(
                        out=junkf, in0=ht, scalar=1.0, in1=ht,
                        op0=mybir.AluOpType.mult, op1=mybir.AluOpType.mult,
                        accum_out=stats[:, C_H2:C_H2 + 1])
                    nc.vector.tensor_scalar(
                        out=junkf, in0=vt, scalar1=1.0, scalar2=None,
                        op0=mybir.AluOpType.mult, op1=mybir.AluOpType.add,
                        accum_out=stats[:, C_V:C_V + 1])
                    nc.vector.tensor_scalar(
                        out=junkf, in0=ht, scalar1=1.0, scalar2=None,
                        op0=mybir.AluOpType.mult, op1=mybir.AluOpType.add,
                        accum_out=stats[:, C_H:C_H + 1])

            # rotate pooled labels into the fat layout via a DRAM bounce
            nc.scalar.dma_start(
                out=d_lpool.rearrange("s a b m -> (a b) s m"), in_=lpool)
            nc.scalar.dma_start(
                out=lfat, in_=d_lpool.rearrange("s a b m -> (s a) (b m)"))

            # attention moments involving l (tail, on DVE)
            nc.vector.scalar_tensor_tensor(
                out=junkf, in0=vt, scalar=1.0, in1=lfat,
                op0=mybir.AluOpType.mult, op1=mybir.AluOpType.mult,
                accum_out=stats[:, C_VL:C_VL + 1])
            nc.vector.scalar_tensor_tensor(
                out=junkf, in0=ht, scalar=1.0, in1=lfat,
                op0=mybir.AluOpType.mult, op1=mybir.AluOpType.mult,
                accum_out=stats[:, C_HL:C_HL + 1])
            nc.vector.scalar_tensor_tensor(
                out=junkf, in0=vh, scalar=1.0, in1=lfat,
                op0=mybir.AluOpType.mult, op1=mybir.AluOpType.mult,
                accum_out=stats[:, C_VHL:C_VHL + 1])
            nc.vector.scalar_tensor_tensor(
                out=junkf, in0=lfat, scalar=1.0, in1=lfat,
                op0=mybir.AluOpType.mult, op1=mybir.AluOpType.mult,
                accum_out=stats[:, C_L2:C_L2 + 1])
            nc.vector.tensor_scalar(
                out=junkf, in0=lfat, scalar1=1.0, scalar2=None,
                op0=mybir.AluOpType.mult, op1=mybir.AluOpType.add,
                accum_out=stats[:, C_L:C_L + 1])

            nc.sync.dma_start(out=d_stats[:, :], in_=stats)
            nc.sync.dma_start(out=d_stats_act[:, :], in_=stats_act)

    nc.compile()
    return nc


_NC_CACHE = None


def _get_nc():
    global _NC_CACHE
    if _NC_CACHE is None:
        _NC_CACHE = _build_nc()
    return _NC_CACHE


def _host_combine(stats_all, stats_act):
    """stats_all: [N_CORES, P, STATS_W] float64 -> scalar loss (float32)."""
    smooth = 1.0
    bce_sum = 0.0
    dice_sum = 0.0
    cor_sum = 0.0
    for i in range(N_CORES):
        st = stats_all[i]
        for s in range(S_PER_CORE):
            lq_sum = stats_act[i, :, s].sum()
            c = C_SQ + 3 * s
            my = st[:, c].sum()
            d1 = st[:, c + 1].sum()
            d2 = st[:, c + 2].sum()
            m_cnt = st[:, C_M + s].sum()
            part = slice(PPS * s, PPS * (s + 1))
            sv = st[part, C_V].sum()
            sh = st[part, C_H].sum()
            svh = st[part, C_VH].sum()
            sv2 = st[part, C_V2].sum()
            sh2 = st[part, C_H2].sum()
            svl = st[part, C_VL].sum()
            shl = st[part, C_HL].sum()
            svhl = st[part, C_VHL].sum()
            sl2 = st[part, C_L2].sum()
            sl = st[part, C_L].sum()

            bce_sum += d1 + lq_sum - d2
            dice_sum += 2.0 * (my + smooth) / (m_cnt + sl + smooth)

            mv, mh, ml = sv / K, sh / K, sl / K
            num = svhl - mv * shl - mh * svl - ml * svh + 2.0 * K * mv * mh * ml
            den = np.sqrt((sv2 - K * mv * mv) * (sh2 - K * mh * mh)
                          * (sl2 - K * ml * ml))
            cor_sum += num / den

    bceloss = -bce_sum / (N * H * W)
    diceloss = 1.0 - dice_sum / N
    cor_loss = -cor_sum / N
    return np.float32(0.2 * bceloss + 0.3 * diceloss + 0.5 * cor_loss)


def _make_in_maps(logits, labels, v_attention, h_attention):
    f16 = np.float16

    # clamp AFTER bf16 rounding so Ln(1-p) never sees exactly 1.0
    pmax = np.float16(1.0 - 2.0 ** -11)
    lg = np.minimum(np.asarray(logits, np.float32).astype(f16), pmax)
    # square layout: row r = 128*t + p  ->  partition p, free t*512+w
    lg = np.ascontiguousarray(
        lg.reshape(N, T, P, W).transpose(0, 2, 1, 3).reshape(N, P, FD))
    lb = np.asarray(labels, np.float32).astype(f16)
    lb = np.ascontiguousarray(
        lb.reshape(N, T, P, W).transpose(0, 2, 1, 3).reshape(N, P, FD))

    # fat attention layout: partition 32*s + a holds rows [4a, 4a+4)
    va = np.asarray(v_attention, np.float32).astype(f16).reshape(N, N2, N2)
    ha = np.asarray(h_attention, np.float32).astype(f16).reshape(N, N2, N2)

    # poolm[p, t, m] = 1 iff m == 32*t + p//4 (row-pool chunk t, offset 32t)
    poolm = np.zeros((P, T, P), dtype=np.float32)
    for t in range(T):
        poolm[np.arange(P), t, 32 * t + np.arange(P) // 4] = 1.0
    poolm = poolm.reshape(P, T * P).astype(f16)
    ident = np.eye(P, dtype=f16)

    in_maps = []
    for i in range(N_CORES):
        sl = slice(i * S_PER_CORE, (i + 1) * S_PER_CORE)
        att = np.empty((P, 2, S_PER_CORE * N2), dtype=f16)
        # att[32s+a, q, :] = {v,h}[4i+s, 4a:4a+4, :] flattened
        att[:, 0, :] = va[sl].reshape(S_PER_CORE * PPS, T * N2)
        att[:, 1, :] = ha[sl].reshape(S_PER_CORE * PPS, T * N2)
        att = np.ascontiguousarray(att.reshape(P, 2 * S_PER_CORE * N2))
        in_maps.append({
            "logits": lg[sl],
            "labels": lb[sl],
            "att": att,
            "poolmat": poolm,
            "ident": ident,
        })
    return in_maps


def kernel(logits, labels, v_attention, h_attention):
    nc = _get_nc()
    in_maps = _make_in_maps(logits, labels, v_attention, h_attention)
    res = run_bass_kernel_spmd(nc, in_maps, core_ids=list(range(N_CORES)))
    stats_all = np.stack(
        [r["stats"].astype(np.float64) for r in res.results], axis=0)
    stats_act = np.stack(
        [r["stats_act"].astype(np.float64) for r in res.results], axis=0)
    return _host_combine(stats_all, stats_act)



# revision 5
# speedup vs baseline: 1.2806x; 1.2806x over previous
"""CovLoss (BCE + Dice + triple-Pearson) Trainium2 Bass kernel, v2.

Data parallel over batch: 32 samples -> 8 cores x 4 samples. Each core
streams fp16 logits/labels once, emits per-partition partial sums; host
combines in float64.

Engine plan (v2, cost-model driven):
  - ACT (~16us): Ln(p), Ln(1-p) [fused accum -> sum(log(1-p))] per sample,
    plus Square/Copy+accum for attention second moments and the PSUM
    collapse (ACT reads PSUM cheaper than DVE).
  - DVE: only ops with perf modes: tensor_scalar (4x) for masks/accums,
    tensor_tensor (2x) for the three big products (D=lnp-lnq, y*D, M*y).
    scalar_tensor_tensor/tensor_tensor_reduce/custom DVE have NO perf modes
    (v1's mistake) and are avoided.
  - PE: row-pool matmuls for label pooling + ones-vector reduces of y*D
    accumulated across samples/chunks into one [1,512] PSUM bank.
  - GPSIMD cannot execute TensorTensor/TensorScalar (walrus rejects) ->
    Pool engine left idle.
  - DMA on SP sequencer only; ~4.4MB/core fp16.
"""

import numpy as np

import concourse.bass as bass
import concourse.bacc as bacc
import concourse.tile as tile
from concourse import mybir
from concourse.bass_utils import run_bass_kernel_spmd

N_CORES = 8
N = 32
S_PER_CORE = N // N_CORES  # 4
H = W = 512
P = 128
T = H // P                 # 4 row blocks
FD = T * W                 # 2048 free elems per partition per sample
N2 = H // 4                # 128 pooled
K = N2 * N2
PPS = P // S_PER_CORE      # 32 partitions per sample in fat layout

F16 = mybir.dt.float16
F32 = mybir.dt.float32

# stats [128, 32] fp32 columns
C_M = 0      # +s : sum(M) per sample
C_MY = 4     # +s : sum(M*y) per sample
C_BCE = 8    # [0, C_BCE]: sum(y*(lnp-lnq)) over all samples (PSUM collapse)
C_V, C_H, C_VH, C_V2, C_H2, C_VL, C_HL, C_VHL, C_L2, C_L = range(10, 20)
STATS_W = 32


def _build_nc():
    nc = bacc.Bacc(trn_type="TRN2")

    d_logits = nc.dram_tensor("logits", [S_PER_CORE, P, FD], F16,
                              kind="ExternalInput")
    d_labels = nc.dram_tensor("labels", [S_PER_CORE, P, FD], F16,
                              kind="ExternalInput")
    d_att = nc.dram_tensor("att", [P, 2 * S_PER_CORE * N2], F16,
                           kind="ExternalInput")
    d_pool = nc.dram_tensor("poolmat", [P, T * P], F16, kind="ExternalInput")

    d_lpool = nc.dram_tensor("lpool", [S_PER_CORE, PPS, T, N2], F16,
                             kind="Internal")
    d_stats = nc.dram_tensor("stats", [P, STATS_W], F32,
                             kind="ExternalOutput")
    d_stats_act = nc.dram_tensor("stats_act", [P, S_PER_CORE], F32,
                                 kind="ExternalOutput")

    with tile.TileContext(nc) as tc:
        with (
            tc.tile_pool(name="consts", bufs=1) as consts,
            tc.tile_pool(name="big", bufs=2) as big,
            tc.tile_pool(name="psum", bufs=2, space="PSUM") as psump,
            tc.tile_pool(name="psbce", bufs=1, space="PSUM") as psbce,
        ):
            stats = consts.tile([P, STATS_W], F32)
            stats_act = consts.tile([P, S_PER_CORE], F32)
            attm = consts.tile([P, 2, S_PER_CORE * N2], F16)
            poolm = consts.tile([P, T, P], F16)
            lpool = consts.tile([P, S_PER_CORE, N2], F16)
            lfat = consts.tile([P, S_PER_CORE * N2], F16)
            vh = consts.tile([P, S_PER_CORE * N2], F16)
            ones = consts.tile([P, 1], F16)
            junkf = consts.tile([P, S_PER_CORE * N2], F16)
            bcejunk = consts.tile([1, W], F32)

            vt = attm[:, 0, :]
            ht = attm[:, 1, :]

            nc.vector.memset(ones, 1.0)
            ps_bce = psbce.tile([1, W], F32)

            for s in range(S_PER_CORE):
                pt = big.tile([P, FD], F16, tag="p")
                nc.sync.dma_start(out=pt, in_=d_logits[s])
                yt = big.tile([P, T, W], F16, tag="y")
                nc.sync.dma_start(
                    out=yt, in_=d_labels[s].rearrange("p (t w) -> p t w", t=T))
                if s == 0:
                    nc.sync.dma_start(
                        out=attm, in_=d_att.rearrange("p (q f) -> p q f", q=2))
                    nc.sync.dma_start(
                        out=poolm, in_=d_pool.rearrange("p (t m) -> p t m",
                                                        t=T))
                ytf = yt.rearrange("p t w -> p (t w)")

                # masks: 4x tensor_scalar, accum -> sum(M) per sample
                mt = big.tile([P, FD], F16, tag="m")
                nc.vector.tensor_scalar(
                    out=mt, in0=pt, scalar1=0.4, scalar2=None,
                    op0=mybir.AluOpType.is_gt, op1=mybir.AluOpType.add,
                    accum_out=stats[:, C_M + s:C_M + s + 1])

                # ACT: the two log passes (engine floor)
                lnp = big.tile([P, FD], F16, tag="lnp")
                nc.scalar.activation(
                    out=lnp, in_=pt, func=mybir.ActivationFunctionType.Ln)
                lnq = big.tile([P, FD], F16, tag="lnq")
                nc.scalar.activation(
                    out=lnq, in_=pt, func=mybir.ActivationFunctionType.Ln,
                    scale=-1.0, bias=1.0,
                    accum_out=stats_act[:, s:s + 1])

                # PE: row-pool matmuls (labels) -> PSUM [128, 512]
                ps_pool = psump.tile([P, W], F32, tag="pool")
                for t in range(T):
                    nc.tensor.matmul(
                        ps_pool, lhsT=poolm[:, t, :], rhs=yt[:, t, :],
                        start=(t == 0), stop=(t == T - 1))

                # DVE 2x products
                dt_ = big.tile([P, FD], F16, tag="d")
                nc.vector.tensor_tensor(
                    out=dt_, in0=lnp, in1=lnq, op=mybir.AluOpType.subtract)
                yd = big.tile([P, T, W], F16, tag="yd")
                ydf = yd.rearrange("p t w -> p (t w)")
                nc.vector.tensor_tensor(
                    out=ydf, in0=ytf, in1=dt_, op=mybir.AluOpType.mult)
                my = big.tile([P, FD], F16, tag="my")
                nc.vector.tensor_tensor(
                    out=my, in0=ytf, in1=mt, op=mybir.AluOpType.mult)
                nc.vector.tensor_scalar(
                    out=my, in0=my, scalar1=1.0, scalar2=None,
                    op0=mybir.AluOpType.mult, op1=mybir.AluOpType.add,
                    accum_out=stats[:, C_MY + s:C_MY + s + 1])

                # PE: ones-reduce of y*D, all chunks+samples into one bank
                for c in range(T):
                    nc.tensor.matmul(
                        ps_bce, lhsT=ones, rhs=yd[:, c, :],
                        start=(s == 0 and c == 0),
                        stop=(s == S_PER_CORE - 1 and c == T - 1),
                        skip_group_check=True)

                # DVE: finish 4x4 pooling (column groups of 4)
                with nc.allow_low_precision(reason="16-term pooled sums"):
                    nc.vector.tensor_reduce(
                        out=lpool[:, s, :],
                        in_=ps_pool.rearrange("p (g f) -> p g f", f=4),
                        axis=mybir.AxisListType.X, op=mybir.AluOpType.add)

                if s == 0:
                    # attention moments needing only v,h (early)
                    nc.vector.tensor_tensor(
                        out=vh, in0=vt, in1=ht, op=mybir.AluOpType.mult)
                    nc.vector.tensor_scalar(
                        out=junkf, in0=vh, scalar1=1.0, scalar2=None,
                        op0=mybir.AluOpType.mult, op1=mybir.AluOpType.add,
                        accum_out=stats[:, C_VH:C_VH + 1])
                    nc.vector.tensor_scalar(
                        out=junkf, in0=vt, scalar1=1.0, scalar2=None,
                        op0=mybir.AluOpType.mult, op1=mybir.AluOpType.add,
                        accum_out=stats[:, C_V:C_V + 1])
                    nc.vector.tensor_scalar(
                        out=junkf, in0=ht, scalar1=1.0, scalar2=None,
                        op0=mybir.AluOpType.mult, op1=mybir.AluOpType.add,
                        accum_out=stats[:, C_H:C_H + 1])

            # ACT extras after the 8 Ln passes: second moments of v,h
            nc.scalar.activation(
                out=junkf, in_=vt, func=mybir.ActivationFunctionType.Square,
                accum_out=stats[:, C_V2:C_V2 + 1])
            nc.scalar.activation(
                out=junkf, in_=ht, func=mybir.ActivationFunctionType.Square,
                accum_out=stats[:, C_H2:C_H2 + 1])

            # rotate pooled labels into fat layout via DRAM bounce
            nc.sync.dma_start(
                out=d_lpool.rearrange("s a b m -> (a b) s m"), in_=lpool)
            nc.sync.dma_start(
                out=lfat, in_=d_lpool.rearrange("s a b m -> (s a) (b m)"))

            # attention moments involving l (tail)
            nc.vector.tensor_tensor(
                out=junkf, in0=vt, in1=lfat, op=mybir.AluOpType.mult)
            nc.vector.tensor_scalar(
                out=junkf, in0=junkf, scalar1=1.0, scalar2=None,
                op0=mybir.AluOpType.mult, op1=mybir.AluOpType.add,
                accum_out=stats[:, C_VL:C_VL + 1])
            nc.vector.tensor_tensor(
                out=junkf, in0=ht, in1=lfat, op=mybir.AluOpType.mult)
            nc.vector.tensor_scalar(
                out=junkf, in0=junkf, scalar1=1.0, scalar2=None,
                op0=mybir.AluOpType.mult, op1=mybir.AluOpType.add,
                accum_out=stats[:, C_HL:C_HL + 1])
            nc.vector.tensor_tensor(
                out=junkf, in0=vh, in1=lfat, op=mybir.AluOpType.mult)
            nc.vector.tensor_scalar(
                out=junkf, in0=junkf, scalar1=1.0, scalar2=None,
                op0=mybir.AluOpType.mult, op1=mybir.AluOpType.add,
                accum_out=stats[:, C_VHL:C_VHL + 1])
            nc.vector.tensor_scalar(
                out=junkf, in0=lfat, scalar1=1.0, scalar2=None,
                op0=mybir.AluOpType.mult, op1=mybir.AluOpType.add,
                accum_out=stats[:, C_L:C_L + 1])
            nc.scalar.activation(
                out=junkf, in_=lfat, func=mybir.ActivationFunctionType.Square,
                accum_out=stats[:, C_L2:C_L2 + 1])

            # collapse the BCE PSUM bank on ACT (cheap PSUM access)
            nc.scalar.activation(
                out=bcejunk, in_=ps_bce,
                func=mybir.ActivationFunctionType.Copy,
                accum_out=stats[0:1, C_BCE:C_BCE + 1])

            nc.sync.dma_start(out=d_stats[:, :], in_=stats)
            nc.sync.dma_start(out=d_stats_act[:, :], in_=stats_act)

    nc.compile()
    return nc


_NC_CACHE = None


def _get_nc():
    global _NC_CACHE
    if _NC_CACHE is None:
        _NC_CACHE = _build_nc()
    return _NC_CACHE


def _host_combine(stats_all, stats_act):
    """stats_all: [N_CORES, P, STATS_W] float64 -> scalar loss (float32)."""
    smooth = 1.0
    bce_sum = 0.0
    dice_sum = 0.0
    cor_sum = 0.0
    for i in range(N_CORES):
        st = stats_all[i]
        bce_sum += st[0, C_BCE] + stats_act[i].sum()
        for s in range(S_PER_CORE):
            my = st[:, C_MY + s].sum()
            m_cnt = st[:, C_M + s].sum()
            part = slice(PPS * s, PPS * (s + 1))
            sv = st[part, C_V].sum()
            sh = st[part, C_H].sum()
            svh = st[part, C_VH].sum()
            sv2 = st[part, C_V2].sum()
            sh2 = st[part, C_H2].sum()
            svl = st[part, C_VL].sum()
            shl = st[part, C_HL].sum()
            svhl = st[part, C_VHL].sum()
            sl2 = st[part, C_L2].sum()
            sl = st[part, C_L].sum()

            dice_sum += 2.0 * (my + smooth) / (m_cnt + sl + smooth)

            mv, mh, ml = sv / K, sh / K, sl / K
            num = svhl - mv * shl - mh * svl - ml * svh + 2.0 * K * mv * mh * ml
            den = np.sqrt((sv2 - K * mv * mv) * (sh2 - K * mh * mh)
                          * (sl2 - K * ml * ml))
            cor_sum += num / den

    bceloss = -bce_sum / (N * H * W)
    diceloss = 1.0 - dice_sum / N
    cor_loss = -cor_sum / N
    return np.float32(0.2 * bceloss + 0.3 * diceloss + 0.5 * cor_loss)


def _make_in_maps(logits, labels, v_attention, h_attention):
    f16 = np.float16

    # clamp AFTER fp16 rounding so Ln(1-p) never sees exactly 1.0
    pmax = np.float16(1.0 - 2.0 ** -11)
    lg = np.minimum(np.asarray(logits, np.float32).astype(f16), pmax)
    # square layout: row r = 128*t + p  ->  partition p, free t*512+w
    lg = np.ascontiguousarray(
        lg.reshape(N, T, P, W).transpose(0, 2, 1, 3).reshape(N, P, FD))
    lb = np.asarray(labels, np.float32).astype(f16)
    lb = np.ascontiguousarray(
        lb.reshape(N, T, P, W).transpose(0, 2, 1, 3).reshape(N, P, FD))

    # fat attention layout: partition 32*s + a holds rows [4a, 4a+4)
    va = np.asarray(v_attention, np.float32).astype(f16).reshape(N, N2, N2)
    ha = np.asarray(h_attention, np.float32).astype(f16).reshape(N, N2, N2)

    # poolm[p, t, m] = 1 iff m == 32*t + p//4 (row-pool chunk t)
    poolm = np.zeros((P, T, P), dtype=np.float32)
    for t in range(T):
        poolm[np.arange(P), t, 32 * t + np.arange(P) // 4] = 1.0
    poolm = poolm.reshape(P, T * P).astype(f16)

    in_maps = []
    for i in range(N_CORES):
        sl = slice(i * S_PER_CORE, (i + 1) * S_PER_CORE)
        att = np.empty((P, 2, S_PER_CORE * N2), dtype=f16)
        att[:, 0, :] = va[sl].reshape(S_PER_CORE * PPS, T * N2)
        att[:, 1, :] = ha[sl].reshape(S_PER_CORE * PPS, T * N2)
        att = np.ascontiguousarray(att.reshape(P, 2 * S_PER_CORE * N2))
        in_maps.append({
            "logits": lg[sl],
            "labels": lb[sl],
            "att": att,
            "poolmat": poolm,
        })
    return in_maps


def kernel(logits, labels, v_attention, h_attention):
    nc = _get_nc()
    in_maps = _make_in_maps(logits, labels, v_attention, h_attention)
    res = run_bass_kernel_spmd(nc, in_maps, core_ids=list(range(N_CORES)))
    stats_all = np.stack(
        [r["stats"].astype(np.float64) for r in res.results], axis=0)
    stats_act = np.stack(
        [r["stats_act"].astype(np.float64) for r in res.results], axis=0)
    return _host_combine(stats_all, stats_act)


# revision 18
# speedup vs baseline: 1.3414x; 1.0474x over previous
"""CovLoss (BCE + Dice + triple-Pearson) Trainium2 Bass kernel, v2.2.

Data parallel over batch: 32 samples -> 8 cores x 4 samples. Each core
streams fp16 logits/labels once, emits per-partition partial sums; host
combines in float64.

Engine plan (cost-model driven):
  - ACT: dummy Ln first (act-table load overlaps the first DMA), then
    Ln(p) / Ln(1-p)+accum per sample (engine floor ~15us), Square+accum
    for v2/h2, and two batched PSUM collapses (Copy+accum over bank rows;
    only rows 0/32/64 are meaningful, the rest is ignored garbage).
  - DVE: only ops with perf modes: tensor_scalar (4x) for masks/accums,
    tensor_tensor (2x) for the big products (D=lnp-lnq, y*D, y*M).
    scalar_tensor_tensor / tensor_tensor_reduce / custom DVE ops have NO
    perf modes (v1's mistake). Pool-finish kept on DVE (batched 2 samples
    per tensor_reduce over a 2-bank PSUM tile).
  - PE: row-pool matmuls (pooling), ones-reduces of y*D (16 matmuls into
    one accumulation row) and of y*M (per-sample rows at partition bases
    0/32/64 - the only legal matmul output bases).
  - GPSIMD can only memset/DMA (walrus rejects its tensor ops).
  - Queue discipline: each engine's emission order matches data readiness
    (in-order sequencers); poolfin after the products, l-moment tail
    interleaved into sample 3.
"""

import numpy as np

import concourse.bass as bass
import concourse.bacc as bacc
import concourse.tile as tile
from concourse import mybir
from concourse.bass_utils import run_bass_kernel_spmd

N_CORES = 8
N = 32
S_PER_CORE = N // N_CORES  # 4
H = W = 512
P = 128
T = H // P                 # 4 row blocks
FD = T * W                 # 2048 free elems per partition per sample
N2 = H // 4                # 128 pooled
K = N2 * N2
PPS = P // S_PER_CORE      # 32 partitions per sample in fat layout

F16 = mybir.dt.float16
F32 = mybir.dt.float32

# stats [128, 32] fp32 columns
C_M = 0      # +s : sum(M) per sample
C_MYP = 4    # rows 0/32/64: sum(M*y) for samples 0..2 (PSUM collapse A)
C_BCP = 5    # row 0: sum(M*y) sample 3; row 32: sum(y*(lnp-lnq)) total
C_V, C_H, C_VH, C_V2, C_H2, C_VL, C_HL, C_VHL, C_L2, C_L = range(10, 20)
STATS_W = 32

ADD = mybir.AluOpType.add
MULT = mybir.AluOpType.mult
LN = mybir.ActivationFunctionType.Ln
SQUARE = mybir.ActivationFunctionType.Square
COPY = mybir.ActivationFunctionType.Copy


def _build_nc():
    nc = bacc.Bacc(trn_type="TRN2")

    d_logits = nc.dram_tensor("logits", [S_PER_CORE, P, FD], F16,
                              kind="ExternalInput")
    d_labels = nc.dram_tensor("labels", [S_PER_CORE, P, FD], F16,
                              kind="ExternalInput")
    d_att = nc.dram_tensor("att", [P, 2 * S_PER_CORE * N2], F16,
                           kind="ExternalInput")
    d_pool = nc.dram_tensor("poolmat", [P, T * P], F16, kind="ExternalInput")

    d_lpool = nc.dram_tensor("lpool", [S_PER_CORE, PPS, T, N2], F16,
                             kind="Internal")
    d_stats = nc.dram_tensor("stats", [P, STATS_W], F32,
                             kind="ExternalOutput")
    d_stats_act = nc.dram_tensor("stats_act", [P, S_PER_CORE], F32,
                                 kind="ExternalOutput")

    with tile.TileContext(nc) as tc:
        with (
            tc.tile_pool(name="consts", bufs=1) as consts,
            tc.tile_pool(name="big", bufs=3) as big,
            tc.tile_pool(name="psum", bufs=2, space="PSUM") as psump,
            tc.tile_pool(name="psred", bufs=1, space="PSUM") as psred,
        ):
            stats = consts.tile([P, STATS_W], F32)
            stats_act = consts.tile([P, S_PER_CORE], F32)
            attm = consts.tile([P, 2, S_PER_CORE * N2], F16)
            poolm = consts.tile([P, T, P], F16)
            lpool = consts.tile([P, S_PER_CORE, N2], F16)
            lfat = consts.tile([P, S_PER_CORE * N2], F16)
            vh = consts.tile([P, S_PER_CORE * N2], F16)
            ones = consts.tile([P, 1], F16)
            zeros65 = consts.tile([P, 65], F16)
            zerosw = consts.tile([P, W], F16)
            bias0 = consts.tile([P, 1], F16)
            bias1 = consts.tile([P, 1], F16)
            junkf = consts.tile([P, S_PER_CORE * N2], F16)
            junkv = consts.tile([P, S_PER_CORE * N2], F16)
            junkw = consts.tile([P, S_PER_CORE * N2], F16)
            cjunkA = consts.tile([65, W], F16)
            cjunkB = consts.tile([33, W], F16)

            vt = attm[:, 0, :]
            ht = attm[:, 1, :]

            nc.vector.memset(ones, 1.0)
            nc.gpsimd.memset(bias0, 0.0)
            nc.gpsimd.memset(bias1, 1.0)

            # dummy Ln: act-table load happens during the first input DMA
            nc.scalar.activation(out=bias1, in_=ones, func=LN, bias=bias0)
            nc.gpsimd.memset(bias1, 1.0)

            bankA = psred.tile([P, W], F32)   # rows 0/32/64: sum(M*y) s0..2
            bankB = psred.tile([P, W], F32)   # row 0: sum(M*y) s3; row 32: bce
            # zero the collapse windows so the Copy+accum reads no garbage
            nc.vector.memset(zeros65, 0.0)
            nc.gpsimd.memset(zerosw, 0.0)
            nc.tensor.matmul(bankA[0:65, :], lhsT=zeros65, rhs=zerosw,
                             start=True, stop=True, skip_group_check=True)
            nc.tensor.matmul(bankB[0:33, :], lhsT=zeros65[:, 0:33], rhs=zerosw,
                             start=True, stop=True, skip_group_check=True)

            # input DMAs up front (SP in-order; p feeds ACT = critical)
            pts, yts = [], []
            for s in range(S_PER_CORE):
                p_tile = big.tile([P, FD], F16, tag=f"p{s}", name=f"p{s}")
                y_tile = big.tile([P, T, W], F16, tag=f"y{s}", name=f"y{s}")
                pts.append(p_tile)
                yts.append(y_tile)
            nc.sync.dma_start(out=pts[0], in_=d_logits[0])
            nc.sync.dma_start(
                out=yts[0], in_=d_labels[0].rearrange("p (t w) -> p t w", t=T))
            nc.sync.dma_start(
                out=attm, in_=d_att.rearrange("p (q f) -> p q f", q=2))
            nc.sync.dma_start(
                out=poolm, in_=d_pool.rearrange("p (t m) -> p t m", t=T))
            for s in (1, 2, 3):
                nc.sync.dma_start(out=pts[s], in_=d_logits[s])
                nc.sync.dma_start(
                    out=yts[s],
                    in_=d_labels[s].rearrange("p (t w) -> p t w", t=T))

            # PE: row-pool matmuls, interleaved with the reduces by readiness
            ps_pools = []
            for pair in range(2):
                ps_pools.append(psump.tile([P, 2 * W], F32, tag="pool",
                                           name=f"pspool{pair}"))

            def emit_pooling(s):
                half = ps_pools[s // 2][:, (s % 2) * W:(s % 2) * W + W]
                for t in range(T):
                    nc.tensor.matmul(
                        half, lhsT=poolm[:, t, :], rhs=yts[s][:, t, :],
                        start=(t == 0), stop=(t == T - 1),
                        skip_group_check=True)

            emit_pooling(0)
            emit_pooling(1)

            for s in range(S_PER_CORE):
                pt, yt = pts[s], yts[s]
                ytf = yt.rearrange("p t w -> p (t w)")

                # mask: 4x tensor_scalar, accum -> sum(M) per sample
                mt = big.tile([P, FD], F16, tag="m")
                nc.vector.tensor_scalar(
                    out=mt, in0=pt, scalar1=0.4, scalar2=None,
                    op0=mybir.AluOpType.is_gt, op1=ADD,
                    accum_out=stats[:, C_M + s:C_M + s + 1])

                if s == 0:
                    # attention moments needing only v,h (fills DVE idle)
                    nc.vector.tensor_tensor(out=vh, in0=vt, in1=ht, op=MULT)
                    nc.vector.tensor_scalar(
                        out=junkf, in0=vh, scalar1=1.0, scalar2=None,
                        op0=MULT, op1=ADD, accum_out=stats[:, C_VH:C_VH + 1])
                    nc.vector.tensor_scalar(
                        out=junkf, in0=vt, scalar1=1.0, scalar2=None,
                        op0=MULT, op1=ADD, accum_out=stats[:, C_V:C_V + 1])
                    nc.vector.tensor_scalar(
                        out=junkf, in0=ht, scalar1=1.0, scalar2=None,
                        op0=MULT, op1=ADD, accum_out=stats[:, C_H:C_H + 1])

                # ACT: the two log passes (engine floor)
                lnp = big.tile([P, FD], F16, tag="lnp")
                nc.scalar.activation(out=lnp, in_=pt, func=LN, bias=bias0)
                lnq = big.tile([P, FD], F16, tag="lnq")
                nc.scalar.activation(
                    out=lnq, in_=pt, func=LN, scale=-1.0, bias=bias1,
                    accum_out=stats_act[:, s:s + 1])

                # DVE 2x products
                dt_ = big.tile([P, FD], F16, tag="d")
                nc.vector.tensor_tensor(
                    out=dt_, in0=lnp, in1=lnq, op=mybir.AluOpType.subtract)
                yd = big.tile([P, T, W], F16, tag="yd")
                nc.vector.tensor_tensor(
                    out=yd.rearrange("p t w -> p (t w)"), in0=ytf, in1=dt_,
                    op=MULT)
                my = big.tile([P, T, W], F16, tag="my")
                nc.vector.tensor_tensor(
                    out=my.rearrange("p t w -> p (t w)"), in0=ytf, in1=mt,
                    op=MULT)

                # PE: ones-reduces. y*D accumulates into bankB row 32 across
                # all samples/chunks; y*M accumulates per sample (rows 0/32/64
                # of bankA, row 0 of bankB - the legal output bases).
                for c in range(T):
                    nc.tensor.matmul(
                        bankB[32:33, :], lhsT=ones, rhs=yd[:, c, :],
                        start=(s == 0 and c == 0),
                        stop=(s == S_PER_CORE - 1 and c == T - 1),
                        skip_group_check=True)
                red = (bankA[PPS * s:PPS * s + 1, :] if s < 3
                       else bankB[0:1, :])
                for c in range(T):
                    nc.tensor.matmul(
                        red, lhsT=ones, rhs=my[:, c, :],
                        start=(c == 0), stop=(c == T - 1),
                        skip_group_check=True)
                if s < 2:
                    emit_pooling(s + 2)

                # DVE: finish 4x4 pooling, two samples per op (2-bank tile)
                if s % 2 == 1:
                    with nc.allow_low_precision(reason="16-term pooled sums"):
                        nc.vector.tensor_reduce(
                            out=lpool.rearrange(
                                "p (u v) m -> p u v m", u=2)[:, s // 2],
                            in_=ps_pools[s // 2].rearrange(
                                "p (v g f) -> p v g f", v=2, f=4),
                            axis=mybir.AxisListType.X, op=ADD)

                if s == S_PER_CORE - 1:
                    # bounce emitted before its readers (Tile deps follow
                    # emission order); poolfins done above
                    nc.sync.dma_start(
                        out=d_lpool.rearrange("s a b m -> (a b) s m"),
                        in_=lpool)
                    nc.sync.dma_start(
                        out=lfat,
                        in_=d_lpool.rearrange("s a b m -> (s a) (b m)"))
                    # l-moment tail on DVE
                    nc.vector.tensor_tensor(
                        out=junkf, in0=vt, in1=lfat, op=MULT)
                    nc.vector.tensor_scalar(
                        out=junkf, in0=junkf, scalar1=1.0, scalar2=None,
                        op0=MULT, op1=ADD,
                        accum_out=stats[:, C_VL:C_VL + 1])
                    nc.vector.tensor_tensor(
                        out=junkf, in0=ht, in1=lfat, op=MULT)
                    nc.vector.tensor_scalar(
                        out=junkf, in0=junkf, scalar1=1.0, scalar2=None,
                        op0=MULT, op1=ADD,
                        accum_out=stats[:, C_HL:C_HL + 1])
                    nc.vector.tensor_tensor(
                        out=junkf, in0=vh, in1=lfat, op=MULT)
                    nc.vector.tensor_scalar(
                        out=junkf, in0=junkf, scalar1=1.0, scalar2=None,
                        op0=MULT, op1=ADD,
                        accum_out=stats[:, C_VHL:C_VHL + 1])
                    nc.vector.tensor_tensor(
                        out=junkf, in0=lfat, in1=lfat, op=MULT)
                    nc.vector.tensor_scalar(
                        out=junkf, in0=junkf, scalar1=1.0, scalar2=None,
                        op0=MULT, op1=ADD,
                        accum_out=stats[:, C_L2:C_L2 + 1])
                    nc.vector.tensor_scalar(
                        out=junkf, in0=lfat, scalar1=1.0, scalar2=None,
                        op0=MULT, op1=ADD,
                        accum_out=stats[:, C_L:C_L + 1])

            # ACT tail: v,h second moments (own junk tiles - no WAR with DVE)
            nc.scalar.activation(
                out=junkv, in_=vt, func=SQUARE, bias=bias0,
                accum_out=stats[:, C_V2:C_V2 + 1])
            nc.scalar.activation(
                out=junkw, in_=ht, func=SQUARE, bias=bias0,
                accum_out=stats[:, C_H2:C_H2 + 1])

            # batched PSUM collapses on ACT: per-partition row sums; host
            # reads only rows 0/32/64 (A) and 0/32 (B)
            nc.scalar.activation(
                out=cjunkA, in_=bankA[0:65, :], func=COPY,
                accum_out=stats[0:65, C_MYP:C_MYP + 1])
            nc.scalar.activation(
                out=cjunkB, in_=bankB[0:33, :], func=COPY,
                accum_out=stats[0:33, C_BCP:C_BCP + 1])

            nc.sync.dma_start(out=d_stats[:, :], in_=stats)
            nc.sync.dma_start(out=d_stats_act[:, :], in_=stats_act)

    nc.compile()
    return nc


_NC_CACHE = None


def _get_nc():
    global _NC_CACHE
    if _NC_CACHE is None:
        _NC_CACHE = _build_nc()
    return _NC_CACHE


def _host_combine(stats_all, stats_act):
    """stats_all: [N_CORES, P, STATS_W] float64 -> scalar loss (float32)."""
    smooth = 1.0
    bce_sum = 0.0
    dice_sum = 0.0
    cor_sum = 0.0
    for i in range(N_CORES):
        st = stats_all[i]
        bce_sum += st[32, C_BCP] + stats_act[i].sum()
        for s in range(S_PER_CORE):
            my = st[PPS * s, C_MYP] if s < 3 else st[0, C_BCP]
            m_cnt = st[:, C_M + s].sum()
            part = slice(PPS * s, PPS * (s + 1))
            sv = st[part, C_V].sum()
            sh = st[part, C_H].sum()
            svh = st[part, C_VH].sum()
            sv2 = st[part, C_V2].sum()
            sh2 = st[part, C_H2].sum()
            svl = st[part, C_VL].sum()
            shl = st[part, C_HL].sum()
            svhl = st[part, C_VHL].sum()
            sl2 = st[part, C_L2].sum()
            sl = st[part, C_L].sum()

            dice_sum += 2.0 * (my + smooth) / (m_cnt + sl + smooth)

            mv, mh, ml = sv / K, sh / K, sl / K
            num = svhl - mv * shl - mh * svl - ml * svh + 2.0 * K * mv * mh * ml
            den = np.sqrt((sv2 - K * mv * mv) * (sh2 - K * mh * mh)
                          * (sl2 - K * ml * ml))
            cor_sum += num / den

    bceloss = -bce_sum / (N * H * W)
    diceloss = 1.0 - dice_sum / N
    cor_loss = -cor_sum / N
    return np.float32(0.2 * bceloss + 0.3 * diceloss + 0.5 * cor_loss)


def _make_in_maps(logits, labels, v_attention, h_attention):
    f16 = np.float16

    # clamp AFTER fp16 rounding so Ln(1-p) never sees exactly 1.0
    pmax = np.float16(1.0 - 2.0 ** -11)
    lg = np.minimum(np.asarray(logits, np.float32).astype(f16), pmax)
    # square layout: row r = 128*t + p  ->  partition p, free t*512+w
    lg = np.ascontiguousarray(
        lg.reshape(N, T, P, W).transpose(0, 2, 1, 3).reshape(N, P, FD))
    lb = np.asarray(labels, np.float32).astype(f16)
    lb = np.ascontiguousarray(
        lb.reshape(N, T, P, W).transpose(0, 2, 1, 3).reshape(N, P, FD))

    # fat attention layout: partition 32*s + a holds rows [4a, 4a+4)
    va = np.asarray(v_attention, np.float32).astype(f16).reshape(N, N2, N2)
    ha = np.asarray(h_attention, np.float32).astype(f16).reshape(N, N2, N2)

    # poolm[p, t, m] = 1 iff m == 32*t + p//4 (row-pool chunk t)
    poolm = np.zeros((P, T, P), dtype=np.float32)
    for t in range(T):
        poolm[np.arange(P), t, 32 * t + np.arange(P) // 4] = 1.0
    poolm = poolm.reshape(P, T * P).astype(f16)

    in_maps = []
    for i in range(N_CORES):
        sl = slice(i * S_PER_CORE, (i + 1) * S_PER_CORE)
        att = np.empty((P, 2, S_PER_CORE * N2), dtype=f16)
        att[:, 0, :] = va[sl].reshape(S_PER_CORE * PPS, T * N2)
        att[:, 1, :] = ha[sl].reshape(S_PER_CORE * PPS, T * N2)
        att = np.ascontiguousarray(att.reshape(P, 2 * S_PER_CORE * N2))
        in_maps.append({
            "logits": lg[sl],
            "labels": lb[sl],
            "att": att,
            "poolmat": poolm,
        })
    return in_maps


def kernel(logits, labels, v_attention, h_attention):
    nc = _get_nc()
    in_maps = _make_in_maps(logits, labels, v_attention, h_attention)
    res = run_bass_kernel_spmd(nc, in_maps, core_ids=list(range(N_CORES)))
    stats_all = np.stack(
        [r["stats"].astype(np.float64) for r in res.results], axis=0)
    stats_act = np.stack(
        [r["stats_act"].astype(np.float64) for r in res.results], axis=0)
    return _host_combine(stats_all, stats_act)


# revision 19
# speedup vs baseline: 1.4205x; 1.0590x over previous
"""CovLoss (BCE + Dice + triple-Pearson) Trainium2 Bass kernel, v2.2.

Data parallel over batch: 32 samples -> 8 cores x 4 samples. Each core
streams fp16 logits/labels once, emits per-partition partial sums; host
combines in float64.

Engine plan (cost-model driven):
  - ACT: dummy Ln first (act-table load overlaps the first DMA), then
    Ln(p) / Ln(1-p)+accum per sample (engine floor ~15us), Square+accum
    for v2/h2, and two batched PSUM collapses (Copy+accum over bank rows;
    only rows 0/32/64 are meaningful, the rest is ignored garbage).
  - DVE: only ops with perf modes: tensor_scalar (4x) for masks/accums,
    tensor_tensor (2x) for the big products (D=lnp-lnq, y*D, y*M).
    scalar_tensor_tensor / tensor_tensor_reduce / custom DVE ops have NO
    perf modes (v1's mistake). Pool-finish kept on DVE (batched 2 samples
    per tensor_reduce over a 2-bank PSUM tile).
  - PE: row-pool matmuls (pooling), ones-reduces of y*D (16 matmuls into
    one accumulation row) and of y*M (per-sample rows at partition bases
    0/32/64 - the only legal matmul output bases).
  - GPSIMD can only memset/DMA (walrus rejects its tensor ops).
  - Queue discipline: each engine's emission order matches data readiness
    (in-order sequencers); poolfin after the products, l-moment tail
    interleaved into sample 3.
"""

import numpy as np

import concourse.bass as bass
import concourse.bacc as bacc
import concourse.tile as tile
from concourse import mybir
from concourse.bass_utils import run_bass_kernel_spmd

N_CORES = 8
N = 32
S_PER_CORE = N // N_CORES  # 4
H = W = 512
P = 128
T = H // P                 # 4 row blocks
FD = T * W                 # 2048 free elems per partition per sample
N2 = H // 4                # 128 pooled
K = N2 * N2
PPS = P // S_PER_CORE      # 32 partitions per sample in fat layout

F16 = mybir.dt.float16
F32 = mybir.dt.float32

# stats [128, 32] fp32 columns
C_M = 0      # +s : sum(M) per sample
C_MYP = 4    # rows 0/32/64: sum(M*y) for samples 0..2 (PSUM collapse A)
C_BCP = 5    # row 0: sum(M*y) sample 3; row 32: sum(y*(lnp-lnq)) total
C_V, C_H, C_VH, C_V2, C_H2, C_VL, C_HL, C_VHL, C_L2, C_L = range(10, 20)
STATS_W = 32

ADD = mybir.AluOpType.add
MULT = mybir.AluOpType.mult
LN = mybir.ActivationFunctionType.Ln
SQUARE = mybir.ActivationFunctionType.Square
COPY = mybir.ActivationFunctionType.Copy


def _build_nc():
    nc = bacc.Bacc(trn_type="TRN2")

    d_logits = nc.dram_tensor("logits", [S_PER_CORE, P, FD], F16,
                              kind="ExternalInput")
    d_labels = nc.dram_tensor("labels", [S_PER_CORE, P, FD], F16,
                              kind="ExternalInput")
    d_att = nc.dram_tensor("att", [P, 2 * S_PER_CORE * N2], F16,
                           kind="ExternalInput")
    d_pool = nc.dram_tensor("poolmat", [P, T * P], F16, kind="ExternalInput")

    d_lpool = nc.dram_tensor("lpool", [S_PER_CORE, PPS, T, N2], F16,
                             kind="Internal")
    d_stats = nc.dram_tensor("stats", [P, STATS_W], F32,
                             kind="ExternalOutput")
    d_stats_act = nc.dram_tensor("stats_act", [P, S_PER_CORE], F32,
                                 kind="ExternalOutput")

    with tile.TileContext(nc) as tc:
        with (
            tc.tile_pool(name="consts", bufs=1) as consts,
            tc.tile_pool(name="big", bufs=3) as big,
            tc.tile_pool(name="psum", bufs=2, space="PSUM") as psump,
            tc.tile_pool(name="psred", bufs=1, space="PSUM") as psred,
        ):
            stats = consts.tile([P, STATS_W], F32)
            stats_act = consts.tile([P, S_PER_CORE], F32)
            attm = consts.tile([P, 2, S_PER_CORE * N2], F16)
            poolm = consts.tile([P, T, P], F16)
            lpool = consts.tile([P, S_PER_CORE, N2], F16)
            lfat = consts.tile([P, S_PER_CORE * N2], F16)
            vh = consts.tile([P, S_PER_CORE * N2], F16)
            ones = consts.tile([P, 1], F16)
            zeros65 = consts.tile([P, 65], F16)
            zerosw = consts.tile([P, W], F16)
            bias0 = consts.tile([P, 1], F16)
            bias1 = consts.tile([P, 1], F16)
            junkf = consts.tile([P, S_PER_CORE * N2], F16)
            junkv = consts.tile([P, S_PER_CORE * N2], F16)
            junkw = consts.tile([P, S_PER_CORE * N2], F16)
            cjunkA = consts.tile([65, W], F16)
            cjunkB = consts.tile([33, W], F16)

            vt = attm[:, 0, :]
            ht = attm[:, 1, :]

            nc.vector.memset(ones, 1.0)
            nc.gpsimd.memset(bias0, 0.0)
            nc.gpsimd.memset(bias1, 1.0)

            # dummy Ln: act-table load happens during the first input DMA
            nc.scalar.activation(out=bias1, in_=ones, func=LN, bias=bias0)
            nc.gpsimd.memset(bias1, 1.0)

            bankA = psred.tile([P, W], F32)   # rows 0/32/64: sum(M*y) s0..2
            bankB = psred.tile([P, W], F32)   # row 0: sum(M*y) s3; row 32: bce
            # zero the collapse windows so the Copy+accum reads no garbage
            nc.vector.memset(zeros65, 0.0)
            nc.gpsimd.memset(zerosw, 0.0)
            nc.tensor.matmul(bankA[0:65, :], lhsT=zeros65, rhs=zerosw,
                             start=True, stop=True, skip_group_check=True)
            nc.tensor.matmul(bankB[0:33, :], lhsT=zeros65[:, 0:33], rhs=zerosw,
                             start=True, stop=True, skip_group_check=True)

            # input DMAs up front (SP in-order; p feeds ACT = critical)
            pts, yts = [], []
            for s in range(S_PER_CORE):
                p_tile = big.tile([P, FD], F16, tag=f"p{s}", name=f"p{s}")
                y_tile = big.tile([P, T, W], F16, tag=f"y{s}", name=f"y{s}")
                pts.append(p_tile)
                yts.append(y_tile)
            nc.sync.dma_start(out=pts[0], in_=d_logits[0])
            nc.sync.dma_start(
                out=yts[0], in_=d_labels[0].rearrange("p (t w) -> p t w", t=T))
            nc.sync.dma_start(
                out=attm, in_=d_att.rearrange("p (q f) -> p q f", q=2))
            nc.sync.dma_start(
                out=poolm, in_=d_pool.rearrange("p (t m) -> p t m", t=T))
            for s in (1, 2, 3):
                nc.sync.dma_start(out=pts[s], in_=d_logits[s])
                nc.sync.dma_start(
                    out=yts[s],
                    in_=d_labels[s].rearrange("p (t w) -> p t w", t=T))

            # PE: row-pool matmuls, interleaved with the reduces by readiness
            ps_pools = []
            for pair in range(2):
                ps_pools.append(psump.tile([P, 2 * W], F32, tag="pool",
                                           name=f"pspool{pair}"))

            def emit_pooling(s):
                half = ps_pools[s // 2][:, (s % 2) * W:(s % 2) * W + W]
                for t in range(T):
                    nc.tensor.matmul(
                        half, lhsT=poolm[:, t, :], rhs=yts[s][:, t, :],
                        start=(t == 0), stop=(t == T - 1),
                        skip_group_check=True)

            emit_pooling(0)
            emit_pooling(1)

            for s in range(S_PER_CORE):
                pt, yt = pts[s], yts[s]
                ytf = yt.rearrange("p t w -> p (t w)")

                # mask: 4x tensor_scalar, accum -> sum(M) per sample
                mt = big.tile([P, FD], F16, tag="m")
                nc.vector.tensor_scalar(
                    out=mt, in0=pt, scalar1=0.4, scalar2=None,
                    op0=mybir.AluOpType.is_gt, op1=ADD,
                    accum_out=stats[:, C_M + s:C_M + s + 1])

                # DVE: finish 4x4 pooling early, two samples per op; feeds
                # the bounce so the l-moment tail can start mid-kernel
                if s % 2 == 1:
                    with nc.allow_low_precision(reason="16-term pooled sums"):
                        nc.vector.tensor_reduce(
                            out=lpool.rearrange(
                                "p (u v) m -> p u v m", u=2)[:, s // 2],
                            in_=ps_pools[s // 2].rearrange(
                                "p (v g f) -> p v g f", v=2, f=4),
                            axis=mybir.AxisListType.X, op=ADD)
                if s == S_PER_CORE - 1:
                    # bounce emitted before its readers (Tile deps follow
                    # emission order)
                    nc.sync.dma_start(
                        out=d_lpool.rearrange("s a b m -> (a b) s m"),
                        in_=lpool)
                    nc.sync.dma_start(
                        out=lfat,
                        in_=d_lpool.rearrange("s a b m -> (s a) (b m)"))

                if s == 0:
                    # attention moments needing only v,h (fills DVE idle)
                    nc.vector.tensor_tensor(out=vh, in0=vt, in1=ht, op=MULT)
                    nc.vector.tensor_scalar(
                        out=junkf, in0=vh, scalar1=1.0, scalar2=None,
                        op0=MULT, op1=ADD, accum_out=stats[:, C_VH:C_VH + 1])
                    nc.vector.tensor_scalar(
                        out=junkf, in0=vt, scalar1=1.0, scalar2=None,
                        op0=MULT, op1=ADD, accum_out=stats[:, C_V:C_V + 1])
                    nc.vector.tensor_scalar(
                        out=junkf, in0=ht, scalar1=1.0, scalar2=None,
                        op0=MULT, op1=ADD, accum_out=stats[:, C_H:C_H + 1])

                # ACT: the two log passes (engine floor)
                lnp = big.tile([P, FD], F16, tag="lnp")
                nc.scalar.activation(out=lnp, in_=pt, func=LN, bias=bias0)
                lnq = big.tile([P, FD], F16, tag="lnq")
                nc.scalar.activation(
                    out=lnq, in_=pt, func=LN, scale=-1.0, bias=bias1,
                    accum_out=stats_act[:, s:s + 1])

                # DVE 2x products
                dt_ = big.tile([P, FD], F16, tag="d")
                nc.vector.tensor_tensor(
                    out=dt_, in0=lnp, in1=lnq, op=mybir.AluOpType.subtract)
                yd = big.tile([P, T, W], F16, tag="yd")
                nc.vector.tensor_tensor(
                    out=yd.rearrange("p t w -> p (t w)"), in0=ytf, in1=dt_,
                    op=MULT)
                my = big.tile([P, T, W], F16, tag="my")
                nc.vector.tensor_tensor(
                    out=my.rearrange("p t w -> p (t w)"), in0=ytf, in1=mt,
                    op=MULT)

                # PE: ones-reduces. y*D accumulates into bankB row 32 across
                # all samples/chunks; y*M accumulates per sample (rows 0/32/64
                # of bankA, row 0 of bankB - the legal output bases).
                for c in range(T):
                    nc.tensor.matmul(
                        bankB[32:33, :], lhsT=ones, rhs=yd[:, c, :],
                        start=(s == 0 and c == 0),
                        stop=(s == S_PER_CORE - 1 and c == T - 1),
                        skip_group_check=True)
                red = (bankA[PPS * s:PPS * s + 1, :] if s < 3
                       else bankB[0:1, :])
                for c in range(T):
                    nc.tensor.matmul(
                        red, lhsT=ones, rhs=my[:, c, :],
                        start=(c == 0), stop=(c == T - 1),
                        skip_group_check=True)
                if s < 2:
                    emit_pooling(s + 2)

                if s == S_PER_CORE - 1:
                    # l-moment tail on DVE (lfat ready via early bounce)
                    nc.vector.tensor_tensor(
                        out=junkf, in0=vt, in1=lfat, op=MULT)
                    nc.vector.tensor_scalar(
                        out=junkf, in0=junkf, scalar1=1.0, scalar2=None,
                        op0=MULT, op1=ADD,
                        accum_out=stats[:, C_VL:C_VL + 1])
                    nc.vector.tensor_tensor(
                        out=junkf, in0=ht, in1=lfat, op=MULT)
                    nc.vector.tensor_scalar(
                        out=junkf, in0=junkf, scalar1=1.0, scalar2=None,
                        op0=MULT, op1=ADD,
                        accum_out=stats[:, C_HL:C_HL + 1])
                    nc.vector.tensor_tensor(
                        out=junkf, in0=vh, in1=lfat, op=MULT)
                    nc.vector.tensor_scalar(
                        out=junkf, in0=junkf, scalar1=1.0, scalar2=None,
                        op0=MULT, op1=ADD,
                        accum_out=stats[:, C_VHL:C_VHL + 1])
                    nc.vector.tensor_tensor(
                        out=junkf, in0=lfat, in1=lfat, op=MULT)
                    nc.vector.tensor_scalar(
                        out=junkf, in0=junkf, scalar1=1.0, scalar2=None,
                        op0=MULT, op1=ADD,
                        accum_out=stats[:, C_L2:C_L2 + 1])
                    nc.vector.tensor_scalar(
                        out=junkf, in0=lfat, scalar1=1.0, scalar2=None,
                        op0=MULT, op1=ADD,
                        accum_out=stats[:, C_L:C_L + 1])

            # ACT tail: v,h second moments (own junk tiles - no WAR with DVE)
            nc.scalar.activation(
                out=junkv, in_=vt, func=SQUARE, bias=bias0,
                accum_out=stats[:, C_V2:C_V2 + 1])
            nc.scalar.activation(
                out=junkw, in_=ht, func=SQUARE, bias=bias0,
                accum_out=stats[:, C_H2:C_H2 + 1])

            # batched PSUM collapses on ACT: per-partition row sums; host
            # reads only rows 0/32/64 (A) and 0/32 (B)
            nc.scalar.activation(
                out=cjunkA, in_=bankA[0:65, :], func=COPY,
                accum_out=stats[0:65, C_MYP:C_MYP + 1])
            nc.scalar.activation(
                out=cjunkB, in_=bankB[0:33, :], func=COPY,
                accum_out=stats[0:33, C_BCP:C_BCP + 1])

            nc.sync.dma_start(out=d_stats[:, :], in_=stats)
            nc.sync.dma_start(out=d_stats_act[:, :], in_=stats_act)

    nc.compile()
    return nc


_NC_CACHE = None


def _get_nc():
    global _NC_CACHE
    if _NC_CACHE is None:
        _NC_CACHE = _build_nc()
    return _NC_CACHE


def _host_combine(stats_all, stats_act):
    """stats_all: [N_CORES, P, STATS_W] float64 -> scalar loss (float32)."""
    smooth = 1.0
    bce_sum = 0.0
    dice_sum = 0.0
    cor_sum = 0.0
    for i in range(N_CORES):
        st = stats_all[i]
        bce_sum += st[32, C_BCP] + stats_act[i].sum()
        for s in range(S_PER_CORE):
            my = st[PPS * s, C_MYP] if s < 3 else st[0, C_BCP]
            m_cnt = st[:, C_M + s].sum()
            part = slice(PPS * s, PPS * (s + 1))
            sv = st[part, C_V].sum()
            sh = st[part, C_H].sum()
            svh = st[part, C_VH].sum()
            sv2 = st[part, C_V2].sum()
            sh2 = st[part, C_H2].sum()
            svl = st[part, C_VL].sum()
            shl = st[part, C_HL].sum()
            svhl = st[part, C_VHL].sum()
            sl2 = st[part, C_L2].sum()
            sl = st[part, C_L].sum()

            dice_sum += 2.0 * (my + smooth) / (m_cnt + sl + smooth)

            mv, mh, ml = sv / K, sh / K, sl / K
            num = svhl - mv * shl - mh * svl - ml * svh + 2.0 * K * mv * mh * ml
            den = np.sqrt((sv2 - K * mv * mv) * (sh2 - K * mh * mh)
                          * (sl2 - K * ml * ml))
            cor_sum += num / den

    bceloss = -bce_sum / (N * H * W)
    diceloss = 1.0 - dice_sum / N
    cor_loss = -cor_sum / N
    return np.float32(0.2 * bceloss + 0.3 * diceloss + 0.5 * cor_loss)


def _make_in_maps(logits, labels, v_attention, h_attention):
    f16 = np.float16

    # clamp AFTER fp16 rounding so Ln(1-p) never sees exactly 1.0
    pmax = np.float16(1.0 - 2.0 ** -11)
    lg = np.minimum(np.asarray(logits, np.float32).astype(f16), pmax)
    # square layout: row r = 128*t + p  ->  partition p, free t*512+w
    lg = np.ascontiguousarray(
        lg.reshape(N, T, P, W).transpose(0, 2, 1, 3).reshape(N, P, FD))
    lb = np.asarray(labels, np.float32).astype(f16)
    lb = np.ascontiguousarray(
        lb.reshape(N, T, P, W).transpose(0, 2, 1, 3).reshape(N, P, FD))

    # fat attention layout: partition 32*s + a holds rows [4a, 4a+4)
    va = np.asarray(v_attention, np.float32).astype(f16).reshape(N, N2, N2)
    ha = np.asarray(h_attention, np.float32).astype(f16).reshape(N, N2, N2)

    # poolm[p, t, m] = 1 iff m == 32*t + p//4 (row-pool chunk t)
    poolm = np.zeros((P, T, P), dtype=np.float32)
    for t in range(T):
        poolm[np.arange(P), t, 32 * t + np.arange(P) // 4] = 1.0
    poolm = poolm.reshape(P, T * P).astype(f16)

    in_maps = []
    for i in range(N_CORES):
        sl = slice(i * S_PER_CORE, (i + 1) * S_PER_CORE)
        att = np.empty((P, 2, S_PER_CORE * N2), dtype=f16)
        att[:, 0, :] = va[sl].reshape(S_PER_CORE * PPS, T * N2)
        att[:, 1, :] = ha[sl].reshape(S_PER_CORE * PPS, T * N2)
        att = np.ascontiguousarray(att.reshape(P, 2 * S_PER_CORE * N2))
        in_maps.append({
            "logits": lg[sl],
            "labels": lb[sl],
            "att": att,
            "poolmat": poolm,
        })
    return in_maps


def kernel(logits, labels, v_attention, h_attention):
    nc = _get_nc()
    in_maps = _make_in_maps(logits, labels, v_attention, h_attention)
    res = run_bass_kernel_spmd(nc, in_maps, core_ids=list(range(N_CORES)))
    stats_all = np.stack(
        [r["stats"].astype(np.float64) for r in res.results], axis=0)
    stats_act = np.stack(
        [r["stats_act"].astype(np.float64) for r in res.results], axis=0)
    return _host_combine(stats_all, stats_act)


# revision 21
# speedup vs baseline: 1.5143x; 1.0660x over previous
"""CovLoss (BCE + Dice + triple-Pearson) Trainium2 Bass kernel, v2.2.

Data parallel over batch: 32 samples -> 8 cores x 4 samples. Each core
streams fp16 logits/labels once, emits per-partition partial sums; host
combines in float64.

Engine plan (cost-model driven):
  - ACT: dummy Ln first (act-table load overlaps the first DMA), then
    Ln(p) / Ln(1-p)+accum per sample (engine floor ~15us), Square+accum
    for v2/h2, and two batched PSUM collapses (Copy+accum over bank rows;
    only rows 0/32/64 are meaningful, the rest is ignored garbage).
  - DVE: only ops with perf modes: tensor_scalar (4x) for masks/accums,
    tensor_tensor (2x) for the big products (D=lnp-lnq, y*D, y*M).
    scalar_tensor_tensor / tensor_tensor_reduce / custom DVE ops have NO
    perf modes (v1's mistake). Pool-finish kept on DVE (batched 2 samples
    per tensor_reduce over a 2-bank PSUM tile).
  - PE: row-pool matmuls (pooling), ones-reduces of y*D (16 matmuls into
    one accumulation row) and of y*M (per-sample rows at partition bases
    0/32/64 - the only legal matmul output bases).
  - GPSIMD can only memset/DMA (walrus rejects its tensor ops).
  - Queue discipline: each engine's emission order matches data readiness
    (in-order sequencers); poolfin after the products, l-moment tail
    interleaved into sample 3.
"""

import numpy as np

import concourse.bass as bass
import concourse.bacc as bacc
import concourse.tile as tile
from concourse import mybir
from concourse.bass_utils import run_bass_kernel_spmd

N_CORES = 8
N = 32
S_PER_CORE = N // N_CORES  # 4
H = W = 512
P = 128
T = H // P                 # 4 row blocks
FD = T * W                 # 2048 free elems per partition per sample
N2 = H // 4                # 128 pooled
K = N2 * N2
PPS = P // S_PER_CORE      # 32 partitions per sample in fat layout

F16 = mybir.dt.float16
F32 = mybir.dt.float32

# stats [128, 32] fp32 columns
C_M = 0      # +s : sum(M) per sample
C_MYP = 4    # rows 0/32/64: sum(M*y) for samples 0..2 (PSUM collapse A)
C_BCP = 5    # row 32: sum(y*(lnp-lnq)) for samples 0..2 (PSUM collapse B)
C_BC3 = 6    # per-partition sum(y*D) sample 3 (DVE accum; PE is cold then)
C_MY3 = 7    # per-partition sum(M*y) sample 3 (DVE accum)
C_V, C_H, C_VH, C_V2, C_H2, C_VL, C_HL, C_VHL, C_L2, C_L = range(10, 20)
STATS_W = 32

ADD = mybir.AluOpType.add
MULT = mybir.AluOpType.mult
LN = mybir.ActivationFunctionType.Ln
SQUARE = mybir.ActivationFunctionType.Square
COPY = mybir.ActivationFunctionType.Copy


def _build_nc():
    nc = bacc.Bacc(trn_type="TRN2")

    d_logits = nc.dram_tensor("logits", [S_PER_CORE, P, FD], F16,
                              kind="ExternalInput")
    d_labels = nc.dram_tensor("labels", [S_PER_CORE, P, FD], F16,
                              kind="ExternalInput")
    d_att = nc.dram_tensor("att", [P, 2 * S_PER_CORE * N2], F16,
                           kind="ExternalInput")
    d_pool = nc.dram_tensor("poolmat", [P, T * P], F16, kind="ExternalInput")

    d_lpool = nc.dram_tensor("lpool", [S_PER_CORE, PPS, T, N2], F16,
                             kind="Internal")
    d_stats = nc.dram_tensor("stats", [P, STATS_W], F32,
                             kind="ExternalOutput")
    d_stats_act = nc.dram_tensor("stats_act", [P, S_PER_CORE], F32,
                                 kind="ExternalOutput")

    with tile.TileContext(nc) as tc:
        with (
            tc.tile_pool(name="consts", bufs=1) as consts,
            tc.tile_pool(name="big", bufs=3) as big,
            tc.tile_pool(name="psum", bufs=2, space="PSUM") as psump,
            tc.tile_pool(name="psred", bufs=1, space="PSUM") as psred,
        ):
            stats = consts.tile([P, STATS_W], F32)
            stats_act = consts.tile([P, S_PER_CORE], F32)
            attm = consts.tile([P, 2, S_PER_CORE * N2], F16)
            poolm = consts.tile([P, T, P], F16)
            lpool = consts.tile([P, S_PER_CORE, N2], F16)
            lfat = consts.tile([P, S_PER_CORE * N2], F16)
            vh = consts.tile([P, S_PER_CORE * N2], F16)
            ones = consts.tile([P, 1], F16)
            zeros65 = consts.tile([P, 65], F16)
            zerosw = consts.tile([P, W], F16)
            bias0 = consts.tile([P, 1], F16)
            bias1 = consts.tile([P, 1], F16)
            junkf = consts.tile([P, S_PER_CORE * N2], F16)
            junkv = consts.tile([P, S_PER_CORE * N2], F16)
            junkw = consts.tile([P, S_PER_CORE * N2], F16)
            cjunkA = consts.tile([65, W], F16)
            cjunkB = consts.tile([33, W], F16)

            vt = attm[:, 0, :]
            ht = attm[:, 1, :]

            nc.vector.memset(ones, 1.0)
            nc.gpsimd.memset(bias0, 0.0)
            nc.gpsimd.memset(bias1, 1.0)

            # dummy Ln: act-table load happens during the first input DMA
            nc.scalar.activation(out=bias1, in_=ones, func=LN, bias=bias0)
            nc.gpsimd.memset(bias1, 1.0)

            bankA = psred.tile([P, W], F32)   # rows 0/32/64: sum(M*y) s0..2
            bankB = psred.tile([P, W], F32)   # row 0: sum(M*y) s3; row 32: bce
            # zero the collapse windows so the Copy+accum reads no garbage
            nc.vector.memset(zeros65, 0.0)
            nc.gpsimd.memset(zerosw, 0.0)
            nc.tensor.matmul(bankA[0:65, :], lhsT=zeros65, rhs=zerosw,
                             start=True, stop=True, skip_group_check=True)
            nc.tensor.matmul(bankB[0:33, :], lhsT=zeros65[:, 0:33], rhs=zerosw,
                             start=True, stop=True, skip_group_check=True)

            # input DMAs up front (SP in-order; p feeds ACT = critical)
            pts, yts = [], []
            for s in range(S_PER_CORE):
                p_tile = big.tile([P, FD], F16, tag=f"p{s}", name=f"p{s}")
                y_tile = big.tile([P, T, W], F16, tag=f"y{s}", name=f"y{s}")
                pts.append(p_tile)
                yts.append(y_tile)
            nc.sync.dma_start(out=pts[0], in_=d_logits[0])
            nc.sync.dma_start(
                out=yts[0], in_=d_labels[0].rearrange("p (t w) -> p t w", t=T))
            nc.sync.dma_start(
                out=attm, in_=d_att.rearrange("p (q f) -> p q f", q=2))
            nc.sync.dma_start(
                out=poolm, in_=d_pool.rearrange("p (t m) -> p t m", t=T))
            for s in (1, 2, 3):
                nc.sync.dma_start(out=pts[s], in_=d_logits[s])
                nc.sync.dma_start(
                    out=yts[s],
                    in_=d_labels[s].rearrange("p (t w) -> p t w", t=T))

            # PE: row-pool matmuls, interleaved with the reduces by readiness
            ps_pools = []
            for pair in range(2):
                ps_pools.append(psump.tile([P, 2 * W], F32, tag="pool",
                                           name=f"pspool{pair}"))

            def emit_pooling(s):
                half = ps_pools[s // 2][:, (s % 2) * W:(s % 2) * W + W]
                for t in range(T):
                    nc.tensor.matmul(
                        half, lhsT=poolm[:, t, :], rhs=yts[s][:, t, :],
                        start=(t == 0), stop=(t == T - 1),
                        skip_group_check=True)

            emit_pooling(0)
            emit_pooling(1)

            def poolfin(pair):
                with nc.allow_low_precision(reason="16-term pooled sums"):
                    nc.vector.tensor_reduce(
                        out=lpool.rearrange(
                            "p (u v) m -> p u v m", u=2)[:, pair],
                        in_=ps_pools[pair].rearrange(
                            "p (v g f) -> p v g f", v=2, f=4),
                        axis=mybir.AxisListType.X, op=ADD)

            for s in range(S_PER_CORE):
                pt, yt = pts[s], yts[s]
                ytf = yt.rearrange("p t w -> p (t w)")

                # mask: 4x tensor_scalar, accum -> sum(M) per sample
                mt = big.tile([P, FD], F16, tag="m")
                nc.vector.tensor_scalar(
                    out=mt, in0=pt, scalar1=0.4, scalar2=None,
                    op0=mybir.AluOpType.is_gt, op1=ADD,
                    accum_out=stats[:, C_M + s:C_M + s + 1])

                # DVE: finish 4x4 pooling for pair 0 in the idle window
                # between is_gt_1 and D_1 (pair 1 goes between yD_2/My_2)
                if s == 1:
                    poolfin(0)

                if s == 0:
                    # attention moments needing only v,h (fills DVE idle)
                    nc.vector.tensor_tensor(out=vh, in0=vt, in1=ht, op=MULT)
                    nc.vector.tensor_scalar(
                        out=junkf, in0=vh, scalar1=1.0, scalar2=None,
                        op0=MULT, op1=ADD, accum_out=stats[:, C_VH:C_VH + 1])
                    nc.vector.tensor_scalar(
                        out=junkf, in0=vt, scalar1=1.0, scalar2=None,
                        op0=MULT, op1=ADD, accum_out=stats[:, C_V:C_V + 1])
                    nc.vector.tensor_scalar(
                        out=junkf, in0=ht, scalar1=1.0, scalar2=None,
                        op0=MULT, op1=ADD, accum_out=stats[:, C_H:C_H + 1])

                # ACT: the two log passes (engine floor)
                lnp = big.tile([P, FD], F16, tag="lnp")
                nc.scalar.activation(out=lnp, in_=pt, func=LN, bias=bias0)
                lnq = big.tile([P, FD], F16, tag="lnq")
                nc.scalar.activation(
                    out=lnq, in_=pt, func=LN, scale=-1.0, bias=bias1,
                    accum_out=stats_act[:, s:s + 1])

                # DVE 2x products
                dt_ = big.tile([P, FD], F16, tag="d")
                nc.vector.tensor_tensor(
                    out=dt_, in0=lnp, in1=lnq, op=mybir.AluOpType.subtract)
                yd = big.tile([P, T, W], F16, tag="yd")
                nc.vector.tensor_tensor(
                    out=yd.rearrange("p t w -> p (t w)"), in0=ytf, in1=dt_,
                    op=MULT)
                if s == 2:
                    poolfin(1)
                    # bounce emitted before its readers (Tile deps follow
                    # emission order)
                    nc.sync.dma_start(
                        out=d_lpool.rearrange("s a b m -> (a b) s m"),
                        in_=lpool)
                    nc.sync.dma_start(
                        out=lfat,
                        in_=d_lpool.rearrange("s a b m -> (s a) (b m)"))
                my = big.tile([P, T, W], F16, tag="my")
                nc.vector.tensor_tensor(
                    out=my.rearrange("p t w -> p (t w)"), in0=ytf, in1=mt,
                    op=MULT)
                if s == S_PER_CORE - 1:
                    # PE is cold by now; reduce sample 3 on DVE instead
                    ydf3 = yd.rearrange("p t w -> p (t w)")
                    nc.vector.tensor_scalar(
                        out=ydf3, in0=ydf3,
                        scalar1=1.0, scalar2=None, op0=MULT, op1=ADD,
                        accum_out=stats[:, C_BC3:C_BC3 + 1])
                    myf3 = my.rearrange("p t w -> p (t w)")
                    nc.vector.tensor_scalar(
                        out=myf3, in0=myf3,
                        scalar1=1.0, scalar2=None, op0=MULT, op1=ADD,
                        accum_out=stats[:, C_MY3:C_MY3 + 1])

                # PE: ones-reduces for samples 0..2 (sample 3 runs on DVE).
                # y*D accumulates into bankB row 32; y*M per sample into
                # bankA rows 0/32/64 (the legal matmul output bases).
                if s < 3:
                    for c in range(T):
                        nc.tensor.matmul(
                            bankB[32:33, :], lhsT=ones, rhs=yd[:, c, :],
                            start=(s == 0 and c == 0),
                            stop=(s == 2 and c == T - 1),
                            skip_group_check=True)
                    for c in range(T):
                        nc.tensor.matmul(
                            bankA[PPS * s:PPS * s + 1, :], lhsT=ones,
                            rhs=my[:, c, :],
                            start=(c == 0), stop=(c == T - 1),
                            skip_group_check=True)
                if s < 2:
                    emit_pooling(s + 2)

                if s == S_PER_CORE - 1:
                    # l-moment tail on DVE (lfat ready via early bounce)
                    nc.vector.tensor_tensor(
                        out=junkf, in0=vt, in1=lfat, op=MULT)
                    nc.vector.tensor_scalar(
                        out=junkf, in0=junkf, scalar1=1.0, scalar2=None,
                        op0=MULT, op1=ADD,
                        accum_out=stats[:, C_VL:C_VL + 1])
                    nc.vector.tensor_tensor(
                        out=junkf, in0=ht, in1=lfat, op=MULT)
                    nc.vector.tensor_scalar(
                        out=junkf, in0=junkf, scalar1=1.0, scalar2=None,
                        op0=MULT, op1=ADD,
                        accum_out=stats[:, C_HL:C_HL + 1])
                    nc.vector.tensor_tensor(
                        out=junkf, in0=vh, in1=lfat, op=MULT)
                    nc.vector.tensor_scalar(
                        out=junkf, in0=junkf, scalar1=1.0, scalar2=None,
                        op0=MULT, op1=ADD,
                        accum_out=stats[:, C_VHL:C_VHL + 1])

            # ACT tail: v,h second moments (own junk tiles - no WAR with DVE)
            nc.scalar.activation(
                out=junkv, in_=vt, func=SQUARE, bias=bias0,
                accum_out=stats[:, C_V2:C_V2 + 1])
            nc.scalar.activation(
                out=junkw, in_=ht, func=SQUARE, bias=bias0,
                accum_out=stats[:, C_H2:C_H2 + 1])

            # batched PSUM collapses on ACT: per-partition row sums; host
            # reads only rows 0/32/64 (A) and 32 (B)
            nc.scalar.activation(
                out=cjunkA, in_=bankA[0:65, :], func=COPY,
                accum_out=stats[0:65, C_MYP:C_MYP + 1])
            nc.scalar.activation(
                out=cjunkB[0:1, :], in_=bankB[32:33, :], func=COPY,
                accum_out=stats[32:33, C_BCP:C_BCP + 1])
            # l moments that need no product ride the idle ACT tail
            nc.scalar.activation(
                out=junkv, in_=lfat, func=SQUARE, bias=bias0,
                accum_out=stats[:, C_L2:C_L2 + 1])
            nc.scalar.activation(
                out=junkw, in_=lfat, func=COPY,
                accum_out=stats[:, C_L:C_L + 1])

            nc.sync.dma_start(out=d_stats[:, :], in_=stats)
            nc.sync.dma_start(out=d_stats_act[:, :], in_=stats_act)

    nc.compile()
    return nc


_NC_CACHE = None


def _get_nc():
    global _NC_CACHE
    if _NC_CACHE is None:
        _NC_CACHE = _build_nc()
    return _NC_CACHE


def _host_combine(stats_all, stats_act):
    """stats_all: [N_CORES, P, STATS_W] float64 -> scalar loss (float32)."""
    smooth = 1.0
    bce_sum = 0.0
    dice_sum = 0.0
    cor_sum = 0.0
    for i in range(N_CORES):
        st = stats_all[i]
        bce_sum += (st[32, C_BCP] + st[:, C_BC3].sum()
                    + stats_act[i].sum())
        for s in range(S_PER_CORE):
            my = st[PPS * s, C_MYP] if s < 3 else st[:, C_MY3].sum()
            m_cnt = st[:, C_M + s].sum()
            part = slice(PPS * s, PPS * (s + 1))
            sv = st[part, C_V].sum()
            sh = st[part, C_H].sum()
            svh = st[part, C_VH].sum()
            sv2 = st[part, C_V2].sum()
            sh2 = st[part, C_H2].sum()
            svl = st[part, C_VL].sum()
            shl = st[part, C_HL].sum()
            svhl = st[part, C_VHL].sum()
            sl2 = st[part, C_L2].sum()
            sl = st[part, C_L].sum()

            dice_sum += 2.0 * (my + smooth) / (m_cnt + sl + smooth)

            mv, mh, ml = sv / K, sh / K, sl / K
            num = svhl - mv * shl - mh * svl - ml * svh + 2.0 * K * mv * mh * ml
            den = np.sqrt((sv2 - K * mv * mv) * (sh2 - K * mh * mh)
                          * (sl2 - K * ml * ml))
            cor_sum += num / den

    bceloss = -bce_sum / (N * H * W)
    diceloss = 1.0 - dice_sum / N
    cor_loss = -cor_sum / N
    return np.float32(0.2 * bceloss + 0.3 * diceloss + 0.5 * cor_loss)


def _make_in_maps(logits, labels, v_attention, h_attention):
    f16 = np.float16

    # clamp AFTER fp16 rounding so Ln(1-p) never sees exactly 1.0
    pmax = np.float16(1.0 - 2.0 ** -11)
    lg = np.minimum(np.asarray(logits, np.float32).astype(f16), pmax)
    # square layout: row r = 128*t + p  ->  partition p, free t*512+w
    lg = np.ascontiguousarray(
        lg.reshape(N, T, P, W).transpose(0, 2, 1, 3).reshape(N, P, FD))
    lb = np.asarray(labels, np.float32).astype(f16)
    lb = np.ascontiguousarray(
        lb.reshape(N, T, P, W).transpose(0, 2, 1, 3).reshape(N, P, FD))

    # fat attention layout: partition 32*s + a holds rows [4a, 4a+4)
    va = np.asarray(v_attention, np.float32).astype(f16).reshape(N, N2, N2)
    ha = np.asarray(h_attention, np.float32).astype(f16).reshape(N, N2, N2)

    # poolm[p, t, m] = 1 iff m == 32*t + p//4 (row-pool chunk t)
    poolm = np.zeros((P, T, P), dtype=np.float32)
    for t in range(T):
        poolm[np.arange(P), t, 32 * t + np.arange(P) // 4] = 1.0
    poolm = poolm.reshape(P, T * P).astype(f16)

    in_maps = []
    for i in range(N_CORES):
        sl = slice(i * S_PER_CORE, (i + 1) * S_PER_CORE)
        att = np.empty((P, 2, S_PER_CORE * N2), dtype=f16)
        att[:, 0, :] = va[sl].reshape(S_PER_CORE * PPS, T * N2)
        att[:, 1, :] = ha[sl].reshape(S_PER_CORE * PPS, T * N2)
        att = np.ascontiguousarray(att.reshape(P, 2 * S_PER_CORE * N2))
        in_maps.append({
            "logits": lg[sl],
            "labels": lb[sl],
            "att": att,
            "poolmat": poolm,
        })
    return in_maps


def kernel(logits, labels, v_attention, h_attention):
    nc = _get_nc()
    in_maps = _make_in_maps(logits, labels, v_attention, h_attention)
    res = run_bass_kernel_spmd(nc, in_maps, core_ids=list(range(N_CORES)))
    stats_all = np.stack(
        [r["stats"].astype(np.float64) for r in res.results], axis=0)
    stats_act = np.stack(
        [r["stats_act"].astype(np.float64) for r in res.results], axis=0)
    return _host_combine(stats_all, stats_act)


# revision 22
# speedup vs baseline: 1.5166x; 1.0015x over previous
"""CovLoss (BCE + Dice + triple-Pearson) Trainium2 Bass kernel, v2.2.

Data parallel over batch: 32 samples -> 8 cores x 4 samples. Each core
streams fp16 logits/labels once, emits per-partition partial sums; host
combines in float64.

Engine plan (cost-model driven):
  - ACT: dummy Ln first (act-table load overlaps the first DMA), then
    Ln(p) / Ln(1-p)+accum per sample (engine floor ~15us), Square+accum
    for v2/h2, and two batched PSUM collapses (Copy+accum over bank rows;
    only rows 0/32/64 are meaningful, the rest is ignored garbage).
  - DVE: only ops with perf modes: tensor_scalar (4x) for masks/accums,
    tensor_tensor (2x) for the big products (D=lnp-lnq, y*D, y*M).
    scalar_tensor_tensor / tensor_tensor_reduce / custom DVE ops have NO
    perf modes (v1's mistake). Pool-finish kept on DVE (batched 2 samples
    per tensor_reduce over a 2-bank PSUM tile).
  - PE: row-pool matmuls (pooling), ones-reduces of y*D (16 matmuls into
    one accumulation row) and of y*M (per-sample rows at partition bases
    0/32/64 - the only legal matmul output bases).
  - GPSIMD can only memset/DMA (walrus rejects its tensor ops).
  - Queue discipline: each engine's emission order matches data readiness
    (in-order sequencers); poolfin after the products, l-moment tail
    interleaved into sample 3.
"""

import numpy as np

import concourse.bass as bass
import concourse.bacc as bacc
import concourse.tile as tile
from concourse import mybir
from concourse.bass_utils import run_bass_kernel_spmd

N_CORES = 8
N = 32
S_PER_CORE = N // N_CORES  # 4
H = W = 512
P = 128
T = H // P                 # 4 row blocks
FD = T * W                 # 2048 free elems per partition per sample
N2 = H // 4                # 128 pooled
K = N2 * N2
PPS = P // S_PER_CORE      # 32 partitions per sample in fat layout

F16 = mybir.dt.float16
F32 = mybir.dt.float32

# stats [128, 32] fp32 columns
C_M = 0      # cols 0..4: sum(M) (sample 0 split in two halves)
C_MYP = 5    # rows 0/32/64: sum(M*y) for samples 0..2 (PSUM collapse A)
C_BCP = 6    # row 32: sum(y*(lnp-lnq)) for samples 0..2 (PSUM collapse B)
C_BC3 = 7    # per-partition sum(y*D) sample 3 (DVE accum; PE is cold then)
C_MY3 = 8    # per-partition sum(M*y) sample 3 (DVE accum)
C_V, C_H, C_VH, C_V2, C_H2, C_VL, C_HL, C_VHL, C_L2, C_L = range(10, 20)
STATS_W = 32

ADD = mybir.AluOpType.add
MULT = mybir.AluOpType.mult
LN = mybir.ActivationFunctionType.Ln
SQUARE = mybir.ActivationFunctionType.Square
COPY = mybir.ActivationFunctionType.Copy


def _build_nc():
    nc = bacc.Bacc(trn_type="TRN2")

    d_logits = nc.dram_tensor("logits", [S_PER_CORE, P, FD], F16,
                              kind="ExternalInput")
    d_labels = nc.dram_tensor("labels", [S_PER_CORE, P, FD], F16,
                              kind="ExternalInput")
    d_att = nc.dram_tensor("att", [P, 2 * S_PER_CORE * N2], F16,
                           kind="ExternalInput")
    d_pool = nc.dram_tensor("poolmat", [P, T * P], F16, kind="ExternalInput")

    d_lpool = nc.dram_tensor("lpool", [S_PER_CORE, PPS, T, N2], F16,
                             kind="Internal")
    d_stats = nc.dram_tensor("stats", [P, STATS_W], F32,
                             kind="ExternalOutput")
    d_stats_act = nc.dram_tensor("stats_act", [P, S_PER_CORE + 1], F32,
                                 kind="ExternalOutput")

    with tile.TileContext(nc) as tc:
        with (
            tc.tile_pool(name="consts", bufs=1) as consts,
            tc.tile_pool(name="big", bufs=3) as big,
            tc.tile_pool(name="psum", bufs=2, space="PSUM") as psump,
            tc.tile_pool(name="psred", bufs=1, space="PSUM") as psred,
        ):
            stats = consts.tile([P, STATS_W], F32)
            stats_act = consts.tile([P, S_PER_CORE + 1], F32)
            attm = consts.tile([P, 2, S_PER_CORE * N2], F16)
            poolm = consts.tile([P, T, P], F16)
            lpool = consts.tile([P, S_PER_CORE, N2], F16)
            lfat = consts.tile([P, S_PER_CORE * N2], F16)
            vh = consts.tile([P, S_PER_CORE * N2], F16)
            ones = consts.tile([P, 1], F16)
            zeros65 = consts.tile([P, 65], F16)
            zerosw = consts.tile([P, W], F16)
            bias0 = consts.tile([P, 1], F16)
            bias1 = consts.tile([P, 1], F16)
            junkf = consts.tile([P, S_PER_CORE * N2], F16)
            junkv = consts.tile([P, S_PER_CORE * N2], F16)
            junkw = consts.tile([P, S_PER_CORE * N2], F16)
            cjunkA = consts.tile([65, W], F16)
            cjunkB = consts.tile([33, W], F16)

            vt = attm[:, 0, :]
            ht = attm[:, 1, :]

            nc.vector.memset(ones, 1.0)
            nc.gpsimd.memset(bias0, 0.0)
            nc.gpsimd.memset(bias1, 1.0)

            # dummy Ln: act-table load happens during the first input DMA
            nc.scalar.activation(out=bias1, in_=ones, func=LN, bias=bias0)
            nc.gpsimd.memset(bias1, 1.0)

            bankA = psred.tile([P, W], F32)   # rows 0/32/64: sum(M*y) s0..2
            bankB = psred.tile([P, W], F32)   # row 0: sum(M*y) s3; row 32: bce
            # zero the collapse windows so the Copy+accum reads no garbage
            nc.vector.memset(zeros65, 0.0)
            nc.gpsimd.memset(zerosw, 0.0)
            nc.tensor.matmul(bankA[0:65, :], lhsT=zeros65, rhs=zerosw,
                             start=True, stop=True, skip_group_check=True)
            nc.tensor.matmul(bankB[0:33, :], lhsT=zeros65[:, 0:33], rhs=zerosw,
                             start=True, stop=True, skip_group_check=True)

            # input DMAs up front (SP in-order; p feeds ACT = critical)
            pts, yts = [], []
            for s in range(S_PER_CORE):
                p_tile = big.tile([P, FD], F16, tag=f"p{s}", name=f"p{s}")
                y_tile = big.tile([P, T, W], F16, tag=f"y{s}", name=f"y{s}")
                pts.append(p_tile)
                yts.append(y_tile)
            nc.sync.dma_start(out=pts[0][:, 0:FD // 2],
                              in_=d_logits[0][:, 0:FD // 2])
            nc.sync.dma_start(out=pts[0][:, FD // 2:FD],
                              in_=d_logits[0][:, FD // 2:FD])
            nc.sync.dma_start(
                out=yts[0], in_=d_labels[0].rearrange("p (t w) -> p t w", t=T))
            nc.sync.dma_start(
                out=attm, in_=d_att.rearrange("p (q f) -> p q f", q=2))
            nc.sync.dma_start(
                out=poolm, in_=d_pool.rearrange("p (t m) -> p t m", t=T))
            for s in (1, 2, 3):
                nc.sync.dma_start(out=pts[s], in_=d_logits[s])
                nc.sync.dma_start(
                    out=yts[s],
                    in_=d_labels[s].rearrange("p (t w) -> p t w", t=T))

            # PE: row-pool matmuls, interleaved with the reduces by readiness
            ps_pools = []
            for pair in range(2):
                ps_pools.append(psump.tile([P, 2 * W], F32, tag="pool",
                                           name=f"pspool{pair}"))

            def emit_pooling(s):
                half = ps_pools[s // 2][:, (s % 2) * W:(s % 2) * W + W]
                for t in range(T):
                    nc.tensor.matmul(
                        half, lhsT=poolm[:, t, :], rhs=yts[s][:, t, :],
                        start=(t == 0), stop=(t == T - 1),
                        skip_group_check=True)

            emit_pooling(0)
            emit_pooling(1)

            def poolfin(pair):
                with nc.allow_low_precision(reason="16-term pooled sums"):
                    nc.vector.tensor_reduce(
                        out=lpool.rearrange(
                            "p (u v) m -> p u v m", u=2)[:, pair],
                        in_=ps_pools[pair].rearrange(
                            "p (v g f) -> p v g f", v=2, f=4),
                        axis=mybir.AxisListType.X, op=ADD)

            for s in range(S_PER_CORE):
                pt, yt = pts[s], yts[s]
                ytf = yt.rearrange("p t w -> p (t w)")

                # mask: 4x tensor_scalar, accum -> sum(M) per sample
                # (sample 0 is split in halves to cut the startup latency;
                # the first half starts as soon as its DMA lands)
                mt = big.tile([P, FD], F16, tag="m")
                if s == 0:
                    nc.vector.tensor_scalar(
                        out=mt[:, 0:FD // 2], in0=pt[:, 0:FD // 2],
                        scalar1=0.4, scalar2=None,
                        op0=mybir.AluOpType.is_gt, op1=ADD,
                        accum_out=stats[:, C_M:C_M + 1])
                    nc.vector.tensor_scalar(
                        out=mt[:, FD // 2:FD], in0=pt[:, FD // 2:FD],
                        scalar1=0.4, scalar2=None,
                        op0=mybir.AluOpType.is_gt, op1=ADD,
                        accum_out=stats[:, C_M + 1:C_M + 2])
                else:
                    nc.vector.tensor_scalar(
                        out=mt, in0=pt, scalar1=0.4, scalar2=None,
                        op0=mybir.AluOpType.is_gt, op1=ADD,
                        accum_out=stats[:, C_M + 1 + s:C_M + 2 + s])

                # DVE: finish 4x4 pooling for pair 0 in the idle window
                # between is_gt_1 and D_1 (pair 1 goes between yD_2/My_2)
                if s == 1:
                    poolfin(0)

                if s == 0:
                    # attention moments needing only v,h (fills DVE idle)
                    nc.vector.tensor_tensor(out=vh, in0=vt, in1=ht, op=MULT)
                    nc.vector.tensor_scalar(
                        out=junkf, in0=vh, scalar1=1.0, scalar2=None,
                        op0=MULT, op1=ADD, accum_out=stats[:, C_VH:C_VH + 1])
                    nc.vector.tensor_scalar(
                        out=junkf, in0=vt, scalar1=1.0, scalar2=None,
                        op0=MULT, op1=ADD, accum_out=stats[:, C_V:C_V + 1])
                    nc.vector.tensor_scalar(
                        out=junkf, in0=ht, scalar1=1.0, scalar2=None,
                        op0=MULT, op1=ADD, accum_out=stats[:, C_H:C_H + 1])

                # ACT: the two log passes (engine floor)
                lnp = big.tile([P, FD], F16, tag="lnp")
                lnq = big.tile([P, FD], F16, tag="lnq")
                if s == 0:
                    h = FD // 2
                    nc.scalar.activation(out=lnp[:, 0:h], in_=pt[:, 0:h],
                                         func=LN, bias=bias0)
                    nc.scalar.activation(
                        out=lnq[:, 0:h], in_=pt[:, 0:h], func=LN,
                        scale=-1.0, bias=bias1,
                        accum_out=stats_act[:, 0:1])
                    nc.scalar.activation(out=lnp[:, h:FD], in_=pt[:, h:FD],
                                         func=LN, bias=bias0)
                    nc.scalar.activation(
                        out=lnq[:, h:FD], in_=pt[:, h:FD], func=LN,
                        scale=-1.0, bias=bias1,
                        accum_out=stats_act[:, S_PER_CORE:S_PER_CORE + 1])
                else:
                    nc.scalar.activation(out=lnp, in_=pt, func=LN, bias=bias0)
                    nc.scalar.activation(
                        out=lnq, in_=pt, func=LN, scale=-1.0, bias=bias1,
                        accum_out=stats_act[:, s:s + 1])

                # DVE 2x products
                dt_ = big.tile([P, FD], F16, tag="d")
                nc.vector.tensor_tensor(
                    out=dt_, in0=lnp, in1=lnq, op=mybir.AluOpType.subtract)
                yd = big.tile([P, T, W], F16, tag="yd")
                nc.vector.tensor_tensor(
                    out=yd.rearrange("p t w -> p (t w)"), in0=ytf, in1=dt_,
                    op=MULT)
                if s == 2:
                    poolfin(1)
                    # bounce emitted before its readers (Tile deps follow
                    # emission order)
                    nc.sync.dma_start(
                        out=d_lpool.rearrange("s a b m -> (a b) s m"),
                        in_=lpool)
                    nc.sync.dma_start(
                        out=lfat,
                        in_=d_lpool.rearrange("s a b m -> (s a) (b m)"))
                my = big.tile([P, T, W], F16, tag="my")
                nc.vector.tensor_tensor(
                    out=my.rearrange("p t w -> p (t w)"), in0=ytf, in1=mt,
                    op=MULT)
                if s == S_PER_CORE - 1:
                    # PE is cold by now; reduce sample 3 on DVE instead
                    ydf3 = yd.rearrange("p t w -> p (t w)")
                    nc.vector.tensor_scalar(
                        out=ydf3, in0=ydf3,
                        scalar1=1.0, scalar2=None, op0=MULT, op1=ADD,
                        accum_out=stats[:, C_BC3:C_BC3 + 1])
                    myf3 = my.rearrange("p t w -> p (t w)")
                    nc.vector.tensor_scalar(
                        out=myf3, in0=myf3,
                        scalar1=1.0, scalar2=None, op0=MULT, op1=ADD,
                        accum_out=stats[:, C_MY3:C_MY3 + 1])

                # PE: ones-reduces for samples 0..2 (sample 3 runs on DVE).
                # y*D accumulates into bankB row 32; y*M per sample into
                # bankA rows 0/32/64 (the legal matmul output bases).
                if s < 3:
                    for c in range(T):
                        nc.tensor.matmul(
                            bankB[32:33, :], lhsT=ones, rhs=yd[:, c, :],
                            start=(s == 0 and c == 0),
                            stop=(s == 2 and c == T - 1),
                            skip_group_check=True)
                    for c in range(T):
                        nc.tensor.matmul(
                            bankA[PPS * s:PPS * s + 1, :], lhsT=ones,
                            rhs=my[:, c, :],
                            start=(c == 0), stop=(c == T - 1),
                            skip_group_check=True)
                if s < 2:
                    emit_pooling(s + 2)

                if s == S_PER_CORE - 1:
                    # l-moment tail on DVE (lfat ready via early bounce)
                    nc.vector.tensor_tensor(
                        out=junkf, in0=vt, in1=lfat, op=MULT)
                    nc.vector.tensor_scalar(
                        out=junkf, in0=junkf, scalar1=1.0, scalar2=None,
                        op0=MULT, op1=ADD,
                        accum_out=stats[:, C_VL:C_VL + 1])
                    nc.vector.tensor_tensor(
                        out=junkf, in0=ht, in1=lfat, op=MULT)
                    nc.vector.tensor_scalar(
                        out=junkf, in0=junkf, scalar1=1.0, scalar2=None,
                        op0=MULT, op1=ADD,
                        accum_out=stats[:, C_HL:C_HL + 1])
                    nc.vector.tensor_tensor(
                        out=junkf, in0=vh, in1=lfat, op=MULT)
                    nc.vector.tensor_scalar(
                        out=junkf, in0=junkf, scalar1=1.0, scalar2=None,
                        op0=MULT, op1=ADD,
                        accum_out=stats[:, C_VHL:C_VHL + 1])

            # ACT tail: v,h second moments (own junk tiles - no WAR with DVE)
            nc.scalar.activation(
                out=junkv, in_=vt, func=SQUARE, bias=bias0,
                accum_out=stats[:, C_V2:C_V2 + 1])
            nc.scalar.activation(
                out=junkw, in_=ht, func=SQUARE, bias=bias0,
                accum_out=stats[:, C_H2:C_H2 + 1])

            # batched PSUM collapses on ACT: per-partition row sums; host
            # reads only rows 0/32/64 (A) and 32 (B)
            nc.scalar.activation(
                out=cjunkA, in_=bankA[0:65, :], func=COPY,
                accum_out=stats[0:65, C_MYP:C_MYP + 1])
            nc.scalar.activation(
                out=cjunkB[0:1, :], in_=bankB[32:33, :], func=COPY,
                accum_out=stats[32:33, C_BCP:C_BCP + 1])
            # l moments that need no product ride the idle ACT tail
            nc.scalar.activation(
                out=junkv, in_=lfat, func=SQUARE, bias=bias0,
                accum_out=stats[:, C_L2:C_L2 + 1])
            nc.scalar.activation(
                out=junkw, in_=lfat, func=COPY,
                accum_out=stats[:, C_L:C_L + 1])

            nc.sync.dma_start(out=d_stats[:, :], in_=stats)
            nc.sync.dma_start(out=d_stats_act[:, :], in_=stats_act)

    nc.compile()
    return nc


_NC_CACHE = None


def _get_nc():
    global _NC_CACHE
    if _NC_CACHE is None:
        _NC_CACHE = _build_nc()
    return _NC_CACHE


def _host_combine(stats_all, stats_act):
    """stats_all: [N_CORES, P, STATS_W] float64 -> scalar loss (float32)."""
    smooth = 1.0
    bce_sum = 0.0
    dice_sum = 0.0
    cor_sum = 0.0
    for i in range(N_CORES):
        st = stats_all[i]
        bce_sum += (st[32, C_BCP] + st[:, C_BC3].sum()
                    + stats_act[i].sum())
        for s in range(S_PER_CORE):
            my = st[PPS * s, C_MYP] if s < 3 else st[:, C_MY3].sum()
            if s == 0:
                m_cnt = st[:, C_M].sum() + st[:, C_M + 1].sum()
            else:
                m_cnt = st[:, C_M + 1 + s].sum()
            part = slice(PPS * s, PPS * (s + 1))
            sv = st[part, C_V].sum()
            sh = st[part, C_H].sum()
            svh = st[part, C_VH].sum()
            sv2 = st[part, C_V2].sum()
            sh2 = st[part, C_H2].sum()
            svl = st[part, C_VL].sum()
            shl = st[part, C_HL].sum()
            svhl = st[part, C_VHL].sum()
            sl2 = st[part, C_L2].sum()
            sl = st[part, C_L].sum()

            dice_sum += 2.0 * (my + smooth) / (m_cnt + sl + smooth)

            mv, mh, ml = sv / K, sh / K, sl / K
            num = svhl - mv * shl - mh * svl - ml * svh + 2.0 * K * mv * mh * ml
            den = np.sqrt((sv2 - K * mv * mv) * (sh2 - K * mh * mh)
                          * (sl2 - K * ml * ml))
            cor_sum += num / den

    bceloss = -bce_sum / (N * H * W)
    diceloss = 1.0 - dice_sum / N
    cor_loss = -cor_sum / N
    return np.float32(0.2 * bceloss + 0.3 * diceloss + 0.5 * cor_loss)


def _make_in_maps(logits, labels, v_attention, h_attention):
    f16 = np.float16

    # clamp AFTER fp16 rounding so Ln(1-p) never sees exactly 1.0
    pmax = np.float16(1.0 - 2.0 ** -11)
    lg = np.minimum(np.asarray(logits, np.float32).astype(f16), pmax)
    # square layout: row r = 128*t + p  ->  partition p, free t*512+w
    lg = np.ascontiguousarray(
        lg.reshape(N, T, P, W).transpose(0, 2, 1, 3).reshape(N, P, FD))
    lb = np.asarray(labels, np.float32).astype(f16)
    lb = np.ascontiguousarray(
        lb.reshape(N, T, P, W).transpose(0, 2, 1, 3).reshape(N, P, FD))

    # fat attention layout: partition 32*s + a holds rows [4a, 4a+4)
    va = np.asarray(v_attention, np.float32).astype(f16).reshape(N, N2, N2)
    ha = np.asarray(h_attention, np.float32).astype(f16).reshape(N, N2, N2)

    # poolm[p, t, m] = 1 iff m == 32*t + p//4 (row-pool chunk t)
    poolm = np.zeros((P, T, P), dtype=np.float32)
    for t in range(T):
        poolm[np.arange(P), t, 32 * t + np.arange(P) // 4] = 1.0
    poolm = poolm.reshape(P, T * P).astype(f16)

    in_maps = []
    for i in range(N_CORES):
        sl = slice(i * S_PER_CORE, (i + 1) * S_PER_CORE)
        att = np.empty((P, 2, S_PER_CORE * N2), dtype=f16)
        att[:, 0, :] = va[sl].reshape(S_PER_CORE * PPS, T * N2)
        att[:, 1, :] = ha[sl].reshape(S_PER_CORE * PPS, T * N2)
        att = np.ascontiguousarray(att.reshape(P, 2 * S_PER_CORE * N2))
        in_maps.append({
            "logits": lg[sl],
            "labels": lb[sl],
            "att": att,
            "poolmat": poolm,
        })
    return in_maps


def kernel(logits, labels, v_attention, h_attention):
    nc = _get_nc()
    in_maps = _make_in_maps(logits, labels, v_attention, h_attention)
    res = run_bass_kernel_spmd(nc, in_maps, core_ids=list(range(N_CORES)))
    stats_all = np.stack(
        [r["stats"].astype(np.float64) for r in res.results], axis=0)
    stats_act = np.stack(
        [r["stats_act"].astype(np.float64) for r in res.results], axis=0)
    return _host_combine(stats_all, stats_act)
